# revision 18
# baseline (speedup 1.0000x reference)
"""Causal multi-head attention (B=2, S=2048, E=2048, H=16, D=128) on 8 TRN2 cores.

Sharding: core c = 4*b + g handles batch b and head-group g (4 heads, feature
slice F = [512g, 512g+512)).  Each core computes q/k/v projections for its
heads, RoPE, causal attention, and a partial output projection
yT_p = Wp[:, F] @ attn_out[F].T.  Host sums the 4 partials per batch and adds
bp.

Schedule: fully software-pipelined around the PE.  The attention j-loop for
tile t weaves in (a) the next tile's q/k/v projection matmuls and (b) the
previous tile's output-projection matmuls, so the in-order PE queue always has
independent work while ScalarE computes exp().

attn@V is computed "flipped" (at stationary, v moving) with a ones column
appended to v, so the softmax denominator falls out of the same matmuls
(column 128 of the PSUM accumulator) instead of costing a second PE pass.
The attention output lands [m, d]; normalization is then a per-partition
scalar multiply on DVE, and a DMA-xbar transpose restores [d, m] for the
output projection.  The causal mask is a 0/1 multiply on DVE (not a PE
matmul).  exp() is done on ScalarE over pairs of score blocks to amortize
instruction overhead.
"""

import math

import ml_dtypes
import numpy as np

import concourse.bass as bass
import concourse.mybir as mybir
import concourse.tile as tile
from concourse import bacc
from concourse.bass_utils import run_bass_kernel_spmd

F32 = mybir.dt.float32
BF16 = mybir.dt.bfloat16

B, S, E, H, D = 2, 2048, 2048, 16, 128
N_CORES = 8
GROUPS = 4          # head-groups per batch
HL = H // GROUPS    # heads per core
BASE = 10000.0


def build_attn_kernel(s=S, e=E, hl=HL, d=D, mt=512, n_cores=N_CORES):
    """One SPMD core program: attention for `hl` heads of one batch."""
    dh = hl * d          # local q/k/v feature width
    et = e // 128        # contraction tiles for the projections
    nmt = s // mt        # m-tiles
    npm = mt // 128      # 128-blocks per m-tile
    ft_out = e // 128    # output g-tiles
    scale = 1.0 / math.sqrt(d)

    nc = bacc.Bacc("TRN2", target_bir_lowering=False, debug=False,
                   num_devices=n_cores)

    xT = nc.dram_tensor("xT", [e, s], BF16, kind="ExternalInput").ap()
    wqT = nc.dram_tensor("wqT", [e, dh], BF16, kind="ExternalInput").ap()
    wkT = nc.dram_tensor("wkT", [e, dh], BF16, kind="ExternalInput").ap()
    wvT = nc.dram_tensor("wvT", [e, dh], BF16, kind="ExternalInput").ap()
    wpT = nc.dram_tensor("wpT", [dh, e], BF16, kind="ExternalInput").ap()
    # bqk columns: [bq | bk | bq rolled by 64 partitions | bk rolled]
    bqk = nc.dram_tensor("bqk", [128, 4 * hl], F32, kind="ExternalInput").ap()
    bv = nc.dram_tensor("bv", [dh], F32, kind="ExternalInput").ap()
    cosT = nc.dram_tensor("cosT", [d, s], BF16, kind="ExternalInput").ap()
    s2T = nc.dram_tensor("s2T", [d, s], BF16, kind="ExternalInput").ap()
    mask = nc.dram_tensor("mask", [128, 128], BF16, kind="ExternalInput").ap()
    ident = nc.dram_tensor("ident", [128, 128], BF16, kind="ExternalInput").ap()
    yT_p = nc.dram_tensor("yT_p", [e, s], BF16, kind="ExternalOutput").ap()

    xT_t = xT.rearrange("(a p) m -> p a m", p=128)
    wq_t = wqT.rearrange("(a p) f -> p a f", p=128)
    wk_t = wkT.rearrange("(a p) f -> p a f", p=128)
    wv_t = wvT.rearrange("(a p) f -> p a f", p=128)

    with tile.TileContext(nc) as tc:
        with (
            tc.tile_pool(name="consts", bufs=1) as consts,
            tc.tile_pool(name="xm", bufs=2) as xm_pool,
            tc.tile_pool(name="kv", bufs=1) as kv_pool,
            tc.tile_pool(name="qm", bufs=2) as qm_pool,
            tc.tile_pool(name="att", bufs=12) as att_pool,
            tc.tile_pool(name="aof", bufs=2) as aof_pool,
            tc.tile_pool(name="ao", bufs=2) as ao_pool,
            tc.tile_pool(name="yo", bufs=3) as yo_pool,
            tc.tile_pool(name="rec", bufs=6) as rec_pool,
            tc.tile_pool(name="pp", bufs=2, space="PSUM") as pp,
            tc.tile_pool(name="psc", bufs=2, space="PSUM") as psc,
            tc.tile_pool(name="pao", bufs=2, space="PSUM") as pao,
        ):
            # ---- startup feed.  sync queue: x tile 0 + small consts + q
            # weights; gpsimd (SWDGE) queue: v/k weights + x tile 1 + p
            # weights.  Chunked so the first projection matmuls can start as
            # soon as the leading chunks land. ----
            # Strict priority order on the two HWDGE queues (they round-robin
            # into the shared DMA engines): criticals first, background last.
            xm_tiles = {}
            xm0 = xm_pool.tile([128, et, mt], BF16, tag="xm")
            xm_tiles[0] = xm0
            wv_sb = consts.tile([128, et, dh], BF16)
            for c0 in range(0, et, 2):
                nc.sync.dma_start(xm0[:, c0:c0 + 2, :], xT_t[:, c0:c0 + 2, 0:mt])
                nc.scalar.dma_start(wv_sb[:, c0:c0 + 2, :], wv_t[:, c0:c0 + 2, :])
            bqk_sb = consts.tile([128, 4 * hl], F32)
            nc.scalar.dma_start(bqk_sb[:], bqk[:])
            bv_sb = consts.tile([128, npm, 128], F32)
            nc.scalar.dma_start(bv_sb[:], bass.AP(
                tensor=bv.tensor, offset=bv.offset, ap=[[0, 128], [1, dh]]))
            mask_sb = consts.tile([128, 128], BF16)
            nc.scalar.dma_start(mask_sb[:], mask[:])
            ident_sb = consts.tile([128, 128], BF16)
            nc.scalar.dma_start(ident_sb[:], ident[:])
            wq_sb = consts.tile([128, et, dh], BF16)
            wk_sb = consts.tile([128, et, dh], BF16)
            for c0 in range(0, et, 2):
                nc.sync.dma_start(wq_sb[:, c0:c0 + 2, :], wq_t[:, c0:c0 + 2, :])
                nc.scalar.dma_start(wk_sb[:, c0:c0 + 2, :], wk_t[:, c0:c0 + 2, :])
            cos_sb = consts.tile([128, s], BF16)
            s2_sb = consts.tile([128, s], BF16)
            nc.sync.dma_start(cos_sb[:], cosT[:])
            nc.sync.dma_start(s2_sb[:], s2T[:])
            # background: x tile 1 (weaves into attention tile 0) and Wp
            xm1 = xm_pool.tile([128, et, mt], BF16, tag="xm")
            xm_tiles[1] = xm1
            for c0 in range(0, et, 4):
                nc.scalar.dma_start(xm1[:, c0:c0 + 4, :],
                                    xT_t[:, c0:c0 + 4, mt:2 * mt])
            wp_sb = consts.tile([128, hl, e], BF16)
            wp_t = wpT.rearrange("(a p) g -> p a g", p=128)
            for hh in range(hl):
                nc.sync.dma_start(wp_sb[:, hh, :], wp_t[:, hh, :])

            kT_sb = kv_pool.tile([128, hl, s], BF16)    # rope'd k, [d, h, n]
            # v with a ones column per (n-block, head): [n_in, j, h, 129]
            v_ext = kv_pool.tile([128, s // 128, hl, 129], BF16)
            nc.vector.memset(v_ext[:, :, :, 128:129], 1.0)
            # zero the score PSUM banks once: paired exp() reads whole banks,
            # including regions no score matmul has written yet
            for _ in range(2):
                zps = psc.tile([128, 2, mt], F32, tag="psc")
                nc.vector.memset(zps[:], 0.0)

            q_tiles = {}
            ao_tiles = {}

            # ---------------- weave machinery ----------------
            pending = []          # list of (deadline, emit_fn); deadline sorts

            def flush(deadline):
                keep = []
                for dl, fn in pending:
                    if dl <= deadline:
                        fn()
                    else:
                        keep.append((dl, fn))
                pending[:] = keep

            pull_acc = [0.0]

            def pull(rate):
                pull_acc[0] += rate
                while pull_acc[0] >= 1.0 and pending:
                    dl, fn = pending.pop(0)
                    fn()
                    pull_acc[0] -= 1.0

            # ---------------- projection units ----------------
            AC = 4                       # contraction chunk per unit

            def proj_units(t):
                """Units for tile t's q/k/v projections (a-chunk major)."""
                units = []
                xm = xm_tiles[t]
                m0 = t * mt
                # v: out rows [m0+nt*128) -> v_ext[:, t*npm+nt, :, 0:128]
                ps_v = {}

                def v_unit(c0, nt):
                    def emit():
                        if c0 == 0:
                            ps_v[nt] = pp.tile([128, npm, 128], F32, tag="pp", name="ps_v")
                        for a in range(c0, c0 + AC):
                            nc.tensor.matmul(
                                ps_v[nt][:], xm[:, a, nt * 128:(nt + 1) * 128],
                                wv_sb[:, a, :], start=(a == 0), stop=(a == et - 1))
                        if c0 + AC == et:
                            j = t * npm + nt
                            nc.vector.tensor_add(out=v_ext[:, j, :, 0:128],
                                                 in0=ps_v[nt][:], in1=bv_sb[:])
                            del ps_v[nt]
                    return emit

                ps_qk = {}

                def qk_unit(which, w_sb, c0, h):
                    def emit():
                        if c0 == 0:
                            ps_qk[(which, h)] = pp.tile([128, mt], F32, tag="pp", name="ps_qk")
                        ps_q = ps_qk[(which, h)]
                        for a in range(c0, c0 + AC):
                            nc.tensor.matmul(
                                ps_q[:], w_sb[:, a, h * 128:(h + 1) * 128],
                                xm[:, a, :], start=(a == 0), stop=(a == et - 1))
                        if c0 + AC == et:
                            bias = bqk_sb[:, which * hl + h:which * hl + h + 1]
                            biasr = bqk_sb[:, 2 * hl + which * hl + h:
                                           2 * hl + which * hl + h + 1]
                            # the three PSUM-reading ops go on DVE (short
                            # queue, fast pp-bank release; the partition-
                            # rolled read is only legal from PSUM); the final
                            # all-SBUF add runs on the idle Pool engine
                            tcos = rec_pool.tile([128, mt], F32, tag="tcos",
                                                 bufs=2)
                            nc.vector.scalar_tensor_tensor(
                                out=tcos[:], in0=ps_q[:], scalar=bias,
                                in1=cos_sb[:, m0:m0 + mt],
                                op0=mybir.AluOpType.add,
                                op1=mybir.AluOpType.mult)
                            u = rec_pool.tile([128, mt], F32, tag="u", bufs=2)
                            nc.vector.scalar_tensor_tensor(
                                out=u[0:64, :], in0=ps_q[64:128, :],
                                scalar=biasr[0:64, :],
                                in1=s2_sb[0:64, m0:m0 + mt],
                                op0=mybir.AluOpType.add,
                                op1=mybir.AluOpType.mult)
                            nc.vector.scalar_tensor_tensor(
                                out=u[64:128, :], in0=ps_q[0:64, :],
                                scalar=biasr[64:128, :],
                                in1=s2_sb[64:128, m0:m0 + mt],
                                op0=mybir.AluOpType.add,
                                op1=mybir.AluOpType.mult)
                            out_ap = (q_tiles[t][:, h, :] if which == 0
                                      else kT_sb[:, h, m0:m0 + mt])
                            nc.gpsimd.tensor_add(out=out_ap, in0=tcos[:],
                                                 in1=u[:])
                            del ps_qk[(which, h)]
                    return emit

                # pair-groups: at most 2 projection PSUM chains in flight
                # (pp pool has bufs=2), while still consuming DMA chunks in
                # arrival order within each group.
                for g in range(npm // 2):
                    for c0 in range(0, et, AC):
                        for nt in (2 * g, 2 * g + 1):
                            units.append(((t, 0), v_unit(c0, nt)))
                for which, w_sb in ((0, wq_sb), (1, wk_sb)):
                    for g in range(hl // 2):
                        for c0 in range(0, et, AC):
                            for h in (2 * g, 2 * g + 1):
                                units.append(
                                    ((t, h), qk_unit(which, w_sb, c0, h)))
                return units

            # ---------------- output-projection units ----------------
            def outproj_units(t_prev, deadline):
                units = []
                m0p = t_prev * mt
                ao_prev = ao_tiles[t_prev]

                def yt_unit(gt):
                    def emit():
                        ps_y = pp.tile([128, mt], F32, tag="pp")
                        for h in range(hl):
                            nc.tensor.matmul(
                                ps_y[:], wp_sb[:, h, gt * 128:(gt + 1) * 128],
                                ao_prev[:, h, :], start=(h == 0),
                                stop=(h == hl - 1))
                        yo = yo_pool.tile([128, mt], BF16, tag="yo")
                        nc.vector.tensor_copy(yo[:], ps_y[:])
                        nc.sync.dma_start(
                            yT_p[gt * 128:(gt + 1) * 128, m0p:m0p + mt], yo[:])
                    return emit

                for gt in range(ft_out):
                    units.append((deadline, yt_unit(gt)))
                return units

            # ---------------- prologue: projections for tile 0 ----------------
            q_tiles[0] = qm_pool.tile([128, hl, mt], BF16, tag="qm", name="q_sb")
            for _, fn in proj_units(0):
                fn()

            # ---------------- main loop ----------------
            # attn@V for head (t,h) is deferred into head (t,h+1)'s pair loop:
            # the PE never waits on exp(), which trails a full head behind.
            deferred = [None]        # (t, h, [at2 per pair], ao tile)

            def emit_attnv_pair(tp, hp, at2_p, p, pao_t):
                for jj in range(2):
                    j = 2 * p + jj
                    r = j - tp * npm
                    for b in range(max(r, 0), npm):
                        half = b // 2
                        if half not in pao_t and j == 0:
                            pao_t[half] = pao.tile([128, 2, 129], F32,
                                                   tag="pao", name="pao_t")
                        # one start=True per PSUM bank: a second start while
                        # the sibling slot's accumulation group is open wipes
                        # the open region (hw bank-scoped start)
                        nc.tensor.matmul(
                            pao_t[half][:, b % 2, :],
                            at2_p[:, jj, b * 128:(b + 1) * 128],
                            v_ext[:, j, hp, :],
                            start=(j == 0 and b % 2 == 0),
                            stop=(j == tp * npm + b),
                            skip_group_check=(b % 2 == 1))

            def evac_head(tp, hp, pao_t, ao_prev):
                # normalize straight out of PSUM + one fused xbar transpose
                aoF = aof_pool.tile([128, npm, 128], BF16, tag="aof",
                                    name="aoF")
                for b in range(npm):
                    pv = pao_t[b // 2]
                    rec = rec_pool.tile([128, 1], F32, tag="rec", name="rec")
                    nc.vector.reciprocal(out=rec[:], in_=pv[:, b % 2, 128:129])
                    nc.vector.tensor_scalar_mul(
                        out=aoF[:, b, :], in0=pv[:, b % 2, 0:128],
                        scalar1=rec[:])
                nc.sync.dma_start_transpose(
                    out=ao_prev[:, hp, :].rearrange("p (b m) -> p b m", b=npm),
                    in_=aoF[:])

            for t in range(nmt):
                nj = (t + 1) * npm
                npairs = nj // 2
                m0 = t * mt
                # x DMA for the projections woven in the NEXT tile
                if t + 2 < nmt:
                    xm_n = xm_pool.tile([128, et, mt], BF16, tag="xm")
                    xm_tiles[t + 2] = xm_n
                    for c0 in range(0, et, 4):
                        nc.gpsimd.dma_start(
                            xm_n[:, c0:c0 + 4, :],
                            xT_t[:, c0:c0 + 4, (t + 2) * mt:(t + 3) * mt])
                if t + 1 < nmt:
                    q_tiles[t + 1] = qm_pool.tile([128, hl, mt], BF16,
                                                  tag="qm", name="q_sb")
                    pending.extend(proj_units(t + 1))

                ao = ao_pool.tile([128, hl, mt], BF16, tag="ao")
                ao_tiles[t] = ao

                for h in range(hl):
                    flush((t, h))
                    if t > 0 and h == 1:
                        # ao(t-1) is complete only after head 0's evac above
                        pending.extend(outproj_units(t - 1, (t, 9)))
                    rate = (len(pending) / float(npairs * (hl - h))
                            if pending else 0.0)
                    prev = deferred[0]
                    pao_t = {}
                    prev_pairs = len(prev[2]) if prev else 0
                    at2s = []
                    for p in range(npairs):
                        ps2 = psc.tile([128, 2, mt], F32, tag="psc")
                        at2 = att_pool.tile([128, 2, mt], BF16, tag="att")
                        for jj in range(2):
                            j = 2 * p + jj
                            r = j - t * npm
                            c0 = max(r, 0) * 128
                            nc.tensor.matmul(
                                ps2[:, jj, c0:],
                                kT_sb[:, h, j * 128:(j + 1) * 128],
                                q_tiles[t][:, h, c0:], start=True,
                                stop=(r < 0))
                            if r >= 0:  # -1e9 on the strictly-upper diagonal
                                nc.tensor.matmul(
                                    ps2[:, jj, r * 128:(r + 1) * 128],
                                    ident_sb[:], mask_sb[:], start=False,
                                    stop=True)
                        nc.scalar.activation(
                            out=at2[:], in_=ps2[:],
                            func=mybir.ActivationFunctionType.Exp, scale=scale)
                        at2s.append(at2)
                        if prev is not None and p < prev_pairs:
                            emit_attnv_pair(prev[0], prev[1], prev[2][p], p,
                                            pao_t)
                            if p == prev_pairs - 1:
                                # all pairs of the deferred head are in; its
                                # evac can go out now (early for cross-tile)
                                evac_head(prev[0], prev[1], pao_t, prev[3])
                        pull(rate)
                    deferred[0] = (t, h, at2s, ao)
                flush((t, 9))

            # ---------------- epilogue ----------------
            # last head's deferred attention, then the final output projection
            prev = deferred[0]
            pao_t = {}
            for p in range(len(prev[2])):
                emit_attnv_pair(prev[0], prev[1], prev[2][p], p, pao_t)
                if p == len(prev[2]) - 1:
                    evac_head(prev[0], prev[1], pao_t, prev[3])
            for _, fn in outproj_units(nmt - 1, (nmt, 9)):
                fn()

    nc.compile()
    return nc


# ---------------------------------------------------------------------------
# host glue
# ---------------------------------------------------------------------------

def _rope_tables_np(s, d):
    inv_freq = 1.0 / (BASE ** (np.arange(0, d, 2, dtype=np.float32) / d))
    t = np.arange(s, dtype=np.float32)
    freqs = np.outer(t, inv_freq)
    emb = np.concatenate([freqs, freqs], axis=-1)          # [S, D]
    return np.cos(emb).astype(np.float32), np.sin(emb).astype(np.float32)


def make_in_maps(x, Wq, bq, Wk, bk, Wv, bv, Wp, s=S, e=E, hl=HL, d=D,
                 groups=GROUPS, b=B):
    bf = ml_dtypes.bfloat16
    dh = hl * d
    cos, sin = _rope_tables_np(s, d)
    cosT = np.ascontiguousarray(cos.T).astype(bf)           # [D, S]
    sgn = np.concatenate([-np.ones(d // 2), np.ones(d // 2)]).astype(np.float32)
    s2T = (np.ascontiguousarray(sin.T) * sgn[:, None]).astype(bf)
    maskv = np.where(np.arange(128)[:, None] <= np.arange(128)[None, :],
                     np.float32(0), np.float32(-1e9)).astype(bf)
    identv = np.eye(128, dtype=bf)
    in_maps = []
    for bi in range(b):
        xT = np.ascontiguousarray(x[bi].T).astype(bf)       # [E, S]
        for g in range(groups):
            fs = slice(g * dh, (g + 1) * dh)
            # bqk layout: column (which*hl + h) = bias for tensor `which`,
            # head h; columns 2*hl.. are the same rolled by 64 partitions
            bqn = np.concatenate([bq[fs].reshape(hl, 128).T,
                                  bk[fs].reshape(hl, 128).T], axis=1)
            bqkv = np.concatenate([bqn, np.roll(bqn, -64, axis=0)], axis=1)
            in_maps.append({
                "xT": xT,
                "wqT": np.ascontiguousarray(Wq[fs, :].T).astype(bf),
                "wkT": np.ascontiguousarray(Wk[fs, :].T).astype(bf),
                "wvT": np.ascontiguousarray(Wv[fs, :].T).astype(bf),
                "wpT": np.ascontiguousarray(Wp[:, fs].T).astype(bf),
                "bqk": np.ascontiguousarray(bqkv).astype(np.float32),
                "bv": np.ascontiguousarray(bv[fs]).astype(np.float32),
                "cosT": cosT,
                "s2T": np.ascontiguousarray(s2T),
                "mask": maskv,
                "ident": identv,
            })
    return in_maps


_NC_CACHE = {}


def _get_kernel():
    key = "full"
    if key not in _NC_CACHE:
        _NC_CACHE[key] = build_attn_kernel()
    return _NC_CACHE[key]


def _run_axon_cached(nc, in_maps):
    """jit once per process; later kernel() calls reuse the compiled runner."""
    import jax
    from jax.sharding import Mesh, PartitionSpec
    from concourse import bass2jax

    if "runner" not in _NC_CACHE:
        bass2jax.install_neuronx_cc_hook()
        n_cores = len(in_maps)
        partition_name = (nc.partition_id_tensor.name
                          if nc.partition_id_tensor else None)
        in_names, out_names, out_avals, zero_outs = [], [], [], []
        for alloc in nc.m.functions[0].allocations:
            if not isinstance(alloc, mybir.MemoryLocationSet):
                continue
            name = alloc.memorylocations[0].name
            if alloc.kind == "ExternalInput":
                if name != partition_name:
                    in_names.append(name)
            elif alloc.kind == "ExternalOutput":
                out_names.append(name)
                shape = tuple(alloc.tensor_shape)
                dtype = mybir.dt.np(alloc.dtype)
                out_avals.append(jax.core.ShapedArray(shape, dtype))
                zero_outs.append(np.zeros(shape, dtype))
        n_params = len(in_names)
        all_in = list(in_names) + out_names + (
            [partition_name] if partition_name else [])

        def _body(*args):
            operands = list(args)
            if partition_name is not None:
                operands.append(bass2jax.partition_id_tensor())
            outs = bass2jax._bass_exec_p.bind(
                *operands, out_avals=tuple(out_avals),
                in_names=tuple(all_in), out_names=tuple(out_names),
                lowering_input_output_aliases=(), sim_require_finite=True,
                sim_require_nnan=True, nc=nc)
            return tuple(outs)

        devices = jax.devices()[:n_cores]
        mesh = Mesh(np.asarray(devices), ("core",))
        in_specs = (PartitionSpec("core"),) * (n_params + len(out_avals))
        out_specs = (PartitionSpec("core"),) * len(out_names)
        fn = jax.jit(jax.shard_map(_body, mesh=mesh, in_specs=in_specs,
                                   out_specs=out_specs, check_rep=False),
                     keep_unused=True)
        _NC_CACHE["runner"] = (fn, in_names, out_names, out_avals, zero_outs,
                               n_cores)
    fn, in_names, out_names, out_avals, zero_outs, n_cores = _NC_CACHE["runner"]
    concat_in = [np.concatenate([np.asarray(m[n]) for m in in_maps], axis=0)
                 for n in in_names]
    concat_zeros = [np.zeros((n_cores * z.shape[0], *z.shape[1:]), z.dtype)
                    for z in zero_outs]
    outs = fn(*concat_in, *concat_zeros)
    return [{n: np.asarray(outs[i]).reshape(n_cores, *out_avals[i].shape)[c]
             for i, n in enumerate(out_names)} for c in range(n_cores)]


def _run(nc, in_maps):
    from concourse._compat import axon_active
    if axon_active():
        try:
            return _run_axon_cached(nc, in_maps)
        except Exception:
            pass  # fall back to the stock path below
    res = run_bass_kernel_spmd(nc, in_maps, core_ids=list(range(len(in_maps))))
    return res.results


def kernel(x, Wq, bq, Wk, bk, Wv, bv, Wp, bp):
    x = np.asarray(x, dtype=np.float32)
    Wq = np.asarray(Wq, np.float32); bq = np.asarray(bq, np.float32)
    Wk = np.asarray(Wk, np.float32); bk = np.asarray(bk, np.float32)
    Wv = np.asarray(Wv, np.float32); bv = np.asarray(bv, np.float32)
    Wp = np.asarray(Wp, np.float32); bp = np.asarray(bp, np.float32)
    nc = _get_kernel()
    in_maps = make_in_maps(x, Wq, bq, Wk, bk, Wv, bv, Wp)
    results = _run(nc, in_maps)
    y = np.empty((B, S, E), np.float32)
    for bi in range(B):
        acc = results[4 * bi + 0]["yT_p"].astype(np.float32).copy()
        for g in range(1, GROUPS):
            acc += results[4 * bi + g]["yT_p"].astype(np.float32)
        y[bi] = acc.T + bp
    return y


# revision 21
# speedup vs baseline: 1.0661x; 1.0661x over previous
"""Causal multi-head attention (B=2, S=2048, E=2048, H=16, D=128) on 8 TRN2 cores.

Sharding: core c = 4*b + g handles batch b and head-group g (4 heads, feature
slice F = [512g, 512g+512)).  Each core computes q/k/v projections for its
heads, RoPE, causal attention, and a partial output projection
yT_p = Wp[:, F] @ attn_out[F].T.  Host sums the 4 partials per batch and adds
bp.

Schedule: fully software-pipelined around the PE.  The attention j-loop for
tile t weaves in (a) the next tile's q/k/v projection matmuls and (b) the
previous tile's output-projection matmuls, so the in-order PE queue always has
independent work while ScalarE computes exp().

attn@V is computed "flipped" (at stationary, v moving) with a ones column
appended to v, so the softmax denominator falls out of the same matmuls
(column 128 of the PSUM accumulator) instead of costing a second PE pass.
The attention output lands [m, d]; normalization is then a per-partition
scalar multiply on DVE, and a DMA-xbar transpose restores [d, m] for the
output projection.  The causal mask is a 0/1 multiply on DVE (not a PE
matmul).  exp() is done on ScalarE over pairs of score blocks to amortize
instruction overhead.
"""

import math

import ml_dtypes
import numpy as np

import concourse.bass as bass
import concourse.mybir as mybir
import concourse.tile as tile
from concourse import bacc
from concourse.bass_utils import run_bass_kernel_spmd

F32 = mybir.dt.float32
BF16 = mybir.dt.bfloat16

B, S, E, H, D = 2, 2048, 2048, 16, 128
N_CORES = 8
GROUPS = 4          # head-groups per batch
HL = H // GROUPS    # heads per core
BASE = 10000.0


def build_attn_kernel(s=S, e=E, hl=HL, d=D, mt=512, n_cores=N_CORES):
    """One SPMD core program: attention for `hl` heads of one batch."""
    dh = hl * d          # local q/k/v feature width
    et = e // 128        # contraction tiles for the projections
    nmt = s // mt        # m-tiles
    npm = mt // 128      # 128-blocks per m-tile
    ft_out = e // 128    # output g-tiles
    scale = 1.0 / math.sqrt(d)

    nc = bacc.Bacc("TRN2", target_bir_lowering=False, debug=False,
                   num_devices=n_cores)

    xT = nc.dram_tensor("xT", [e, s], BF16, kind="ExternalInput").ap()
    wqT = nc.dram_tensor("wqT", [e, dh], BF16, kind="ExternalInput").ap()
    wkT = nc.dram_tensor("wkT", [e, dh], BF16, kind="ExternalInput").ap()
    wvT = nc.dram_tensor("wvT", [e, dh], BF16, kind="ExternalInput").ap()
    wpT = nc.dram_tensor("wpT", [dh, e], BF16, kind="ExternalInput").ap()
    # bqk columns: [bq | bk | bq rolled by 64 partitions | bk rolled]
    bqk = nc.dram_tensor("bqk", [128, 4 * hl], F32, kind="ExternalInput").ap()
    bv = nc.dram_tensor("bv", [dh], F32, kind="ExternalInput").ap()
    cosT = nc.dram_tensor("cosT", [d, s], BF16, kind="ExternalInput").ap()
    s2T = nc.dram_tensor("s2T", [d, s], BF16, kind="ExternalInput").ap()
    mask = nc.dram_tensor("mask", [128, 128], BF16, kind="ExternalInput").ap()
    ident = nc.dram_tensor("ident", [128, 128], BF16, kind="ExternalInput").ap()
    yT_p = nc.dram_tensor("yT_p", [e, s], BF16, kind="ExternalOutput").ap()

    xT_t = xT.rearrange("(a p) m -> p a m", p=128)
    wq_t = wqT.rearrange("(a p) f -> p a f", p=128)
    wk_t = wkT.rearrange("(a p) f -> p a f", p=128)
    wv_t = wvT.rearrange("(a p) f -> p a f", p=128)

    with tile.TileContext(nc) as tc:
        with (
            tc.tile_pool(name="consts", bufs=1) as consts,
            tc.tile_pool(name="xm", bufs=2) as xm_pool,
            tc.tile_pool(name="kv", bufs=1) as kv_pool,
            tc.tile_pool(name="qm", bufs=2) as qm_pool,
            tc.tile_pool(name="att", bufs=12) as att_pool,
            tc.tile_pool(name="aof", bufs=2) as aof_pool,
            tc.tile_pool(name="ao", bufs=2) as ao_pool,
            tc.tile_pool(name="yo", bufs=3) as yo_pool,
            tc.tile_pool(name="rec", bufs=6) as rec_pool,
            tc.tile_pool(name="pp", bufs=2, space="PSUM") as pp,
            tc.tile_pool(name="psc", bufs=2, space="PSUM") as psc,
            tc.tile_pool(name="pao", bufs=2, space="PSUM") as pao,
        ):
            # ---- startup feed.  sync queue: x tile 0 + small consts + q
            # weights; gpsimd (SWDGE) queue: v/k weights + x tile 1 + p
            # weights.  Chunked so the first projection matmuls can start as
            # soon as the leading chunks land. ----
            # Strict priority order on the two HWDGE queues (they round-robin
            # into the shared DMA engines): criticals first, background last.
            xm_tiles = {}
            xm0 = xm_pool.tile([128, et, mt], BF16, tag="xm")
            xm_tiles[0] = xm0
            wv_sb = consts.tile([128, et, dh], BF16)
            for c0 in range(0, et, 2):
                nc.sync.dma_start(xm0[:, c0:c0 + 2, :], xT_t[:, c0:c0 + 2, 0:mt])
                nc.scalar.dma_start(wv_sb[:, c0:c0 + 2, :], wv_t[:, c0:c0 + 2, :])
            bqk_sb = consts.tile([128, 4 * hl], F32)
            nc.scalar.dma_start(bqk_sb[:], bqk[:])
            bv_sb = consts.tile([128, npm, 128], F32)
            nc.scalar.dma_start(bv_sb[:], bass.AP(
                tensor=bv.tensor, offset=bv.offset, ap=[[0, 128], [1, dh]]))
            mask_sb = consts.tile([128, 128], BF16)
            nc.scalar.dma_start(mask_sb[:], mask[:])
            ident_sb = consts.tile([128, 128], BF16)
            nc.scalar.dma_start(ident_sb[:], ident[:])
            wq_sb = consts.tile([128, et, dh], BF16)
            wk_sb = consts.tile([128, et, dh], BF16)
            for c0 in range(0, et, 2):
                nc.sync.dma_start(wq_sb[:, c0:c0 + 2, :], wq_t[:, c0:c0 + 2, :])
                nc.scalar.dma_start(wk_sb[:, c0:c0 + 2, :], wk_t[:, c0:c0 + 2, :])
            cos_sb = consts.tile([128, s], BF16)
            s2_sb = consts.tile([128, s], BF16)
            nc.sync.dma_start(cos_sb[:], cosT[:])
            nc.sync.dma_start(s2_sb[:], s2T[:])
            # background: x tile 1 (weaves into attention tile 0) and Wp
            xm1 = xm_pool.tile([128, et, mt], BF16, tag="xm")
            xm_tiles[1] = xm1
            for c0 in range(0, et, 4):
                nc.scalar.dma_start(xm1[:, c0:c0 + 4, :],
                                    xT_t[:, c0:c0 + 4, mt:2 * mt])
            wp_sb = consts.tile([128, hl, e], BF16)
            wp_t = wpT.rearrange("(a p) g -> p a g", p=128)
            for hh in range(hl):
                nc.sync.dma_start(wp_sb[:, hh, :], wp_t[:, hh, :])

            kT_sb = kv_pool.tile([128, hl, s], BF16)    # rope'd k, [d, h, n]
            # v with a ones column per (n-block, head): [n_in, j, h, 129]
            v_ext = kv_pool.tile([128, s // 128, hl, 129], BF16)
            nc.vector.memset(v_ext[:, :, :, 128:129], 1.0)
            # zero the score PSUM banks once: paired exp() reads whole banks,
            # including regions no score matmul has written yet
            for _ in range(2):
                zps = psc.tile([128, 2, mt], F32, tag="psc")
                nc.vector.memset(zps[:], 0.0)

            q_tiles = {}
            ao_tiles = {}

            # ---------------- weave machinery ----------------
            pending = []          # list of (deadline, emit_fn); deadline sorts

            def flush(deadline):
                keep = []
                for dl, fn in pending:
                    if dl <= deadline:
                        fn()
                    else:
                        keep.append((dl, fn))
                pending[:] = keep

            pull_acc = [0.0]

            def pull(rate):
                pull_acc[0] += rate
                while pull_acc[0] >= 1.0 and pending:
                    dl, fn = pending.pop(0)
                    fn()
                    pull_acc[0] -= 1.0

            # ---------------- projection units ----------------
            AC = 4                       # contraction chunk per unit

            def proj_units(t):
                """Units for tile t's q/k/v projections (a-chunk major)."""
                units = []
                xm = xm_tiles[t]
                m0 = t * mt
                # v: out rows [m0+nt*128) -> v_ext[:, t*npm+nt, :, 0:128]
                ps_v = {}

                def v_unit(c0, nt):
                    def emit():
                        if c0 == 0:
                            ps_v[nt] = pp.tile([128, npm, 128], F32, tag="pp", name="ps_v")
                        for a in range(c0, c0 + AC):
                            nc.tensor.matmul(
                                ps_v[nt][:], xm[:, a, nt * 128:(nt + 1) * 128],
                                wv_sb[:, a, :], start=(a == 0), stop=(a == et - 1))
                        if c0 + AC == et:
                            j = t * npm + nt
                            nc.vector.tensor_add(out=v_ext[:, j, :, 0:128],
                                                 in0=ps_v[nt][:], in1=bv_sb[:])
                            del ps_v[nt]
                    return emit

                ps_qk = {}

                def qk_unit(which, w_sb, c0, h):
                    def emit():
                        if c0 == 0:
                            ps_qk[(which, h)] = pp.tile([128, mt], F32, tag="pp", name="ps_qk")
                        ps_q = ps_qk[(which, h)]
                        for a in range(c0, c0 + AC):
                            nc.tensor.matmul(
                                ps_q[:], w_sb[:, a, h * 128:(h + 1) * 128],
                                xm[:, a, :], start=(a == 0), stop=(a == et - 1))
                        if c0 + AC == et:
                            bias = bqk_sb[:, which * hl + h:which * hl + h + 1]
                            biasr = bqk_sb[:, 2 * hl + which * hl + h:
                                           2 * hl + which * hl + h + 1]
                            # the three PSUM-reading ops go on DVE (short
                            # queue, fast pp-bank release; the partition-
                            # rolled read is only legal from PSUM); the final
                            # all-SBUF add runs on the idle Pool engine
                            tcos = rec_pool.tile([128, mt], F32, tag="tcos",
                                                 bufs=2)
                            nc.vector.scalar_tensor_tensor(
                                out=tcos[:], in0=ps_q[:], scalar=bias,
                                in1=cos_sb[:, m0:m0 + mt],
                                op0=mybir.AluOpType.add,
                                op1=mybir.AluOpType.mult)
                            u = rec_pool.tile([128, mt], F32, tag="u", bufs=2)
                            nc.vector.scalar_tensor_tensor(
                                out=u[0:64, :], in0=ps_q[64:128, :],
                                scalar=biasr[0:64, :],
                                in1=s2_sb[0:64, m0:m0 + mt],
                                op0=mybir.AluOpType.add,
                                op1=mybir.AluOpType.mult)
                            nc.vector.scalar_tensor_tensor(
                                out=u[64:128, :], in0=ps_q[0:64, :],
                                scalar=biasr[64:128, :],
                                in1=s2_sb[64:128, m0:m0 + mt],
                                op0=mybir.AluOpType.add,
                                op1=mybir.AluOpType.mult)
                            out_ap = (q_tiles[t][:, h, :] if which == 0
                                      else kT_sb[:, h, m0:m0 + mt])
                            nc.gpsimd.tensor_add(out=out_ap, in0=tcos[:],
                                                 in1=u[:])
                            del ps_qk[(which, h)]
                    return emit

                # lagged interleave: exactly 2 chains in flight (pp pool
                # has 2 banks), with consecutive chain-ends a full unit
                # apart so each chain's PSUM-reading tail ops can finish
                # before its bank is reallocated.
                def lace(chains):
                    # chains: list of (deadline, [unit...]); emit with lag
                    out, nu = [], et // AC
                    seq = []
                    for ci, (dl, us) in enumerate(chains):
                        for ui, u_ in enumerate(us):
                            seq.append((2 * ui + 3 * ci, ci, (dl, u_)))
                    seq.sort(key=lambda x: (x[0], x[1]))
                    return [x[2] for x in seq]

                vchains = [((t, 0), [v_unit(c0, nt)
                                     for c0 in range(0, et, AC)])
                           for nt in range(npm)]
                units.extend(lace(vchains))
                for which, w_sb in ((0, wq_sb), (1, wk_sb)):
                    qchains = [((t, h), [qk_unit(which, w_sb, c0, h)
                                         for c0 in range(0, et, AC)])
                               for h in range(hl)]
                    units.extend(lace(qchains))
                return units

            # ---------------- output-projection units ----------------
            def outproj_units(t_prev, deadline):
                units = []
                m0p = t_prev * mt
                ao_prev = ao_tiles[t_prev]
                # woven at tile t_prev+1: Act is exp-loaded at the last tile,
                # so those units evacuate via DVE; the epilogue also spreads
                # its output DMAs over both HWDGE queues
                on_dve = (t_prev == nmt - 2)
                epilogue = (t_prev == nmt - 1)

                def yt_unit(gt):
                    def emit():
                        ps_y = pp.tile([128, mt], F32, tag="pp")
                        for h in range(hl):
                            nc.tensor.matmul(
                                ps_y[:], wp_sb[:, h, gt * 128:(gt + 1) * 128],
                                ao_prev[:, h, :], start=(h == 0),
                                stop=(h == hl - 1))
                        yo = yo_pool.tile([128, mt], BF16, tag="yo")
                        if on_dve:
                            nc.vector.tensor_copy(yo[:], ps_y[:])
                        else:
                            nc.scalar.copy(out=yo[:], in_=ps_y[:])
                        q = (nc.scalar if (epilogue and gt % 2) else nc.sync)
                        q.dma_start(
                            yT_p[gt * 128:(gt + 1) * 128, m0p:m0p + mt], yo[:])
                    return emit

                for gt in range(ft_out):
                    units.append((deadline, yt_unit(gt)))
                return units

            # ---------------- prologue: projections for tile 0 ----------------
            q_tiles[0] = qm_pool.tile([128, hl, mt], BF16, tag="qm", name="q_sb")
            for _, fn in proj_units(0):
                fn()

            # ---------------- main loop ----------------
            # attn@V for head (t,h) is deferred into head (t,h+1)'s pair loop:
            # the PE never waits on exp(), which trails a full head behind.
            deferred = [None]        # (t, h, [at2 per pair], ao tile)

            def emit_attnv_pair(tp, hp, at2_p, p, pao_t):
                for jj in range(2):
                    j = 2 * p + jj
                    r = j - tp * npm
                    for b in range(max(r, 0), npm):
                        half = b // 2
                        if half not in pao_t and j == 0:
                            pao_t[half] = pao.tile([128, 2, 129], F32,
                                                   tag="pao", name="pao_t")
                        # one start=True per PSUM bank: a second start while
                        # the sibling slot's accumulation group is open wipes
                        # the open region (hw bank-scoped start)
                        nc.tensor.matmul(
                            pao_t[half][:, b % 2, :],
                            at2_p[:, jj, b * 128:(b + 1) * 128],
                            v_ext[:, j, hp, :],
                            start=(j == 0 and b % 2 == 0),
                            stop=(j == tp * npm + b),
                            skip_group_check=(b % 2 == 1))

            def evac_head(tp, hp, pao_t, ao_prev):
                # normalize straight out of PSUM + one fused xbar transpose
                aoF = aof_pool.tile([128, npm, 128], BF16, tag="aof",
                                    name="aoF")
                for b in range(npm):
                    pv = pao_t[b // 2]
                    rec = rec_pool.tile([128, 1], F32, tag="rec", name="rec")
                    nc.vector.reciprocal(out=rec[:], in_=pv[:, b % 2, 128:129])
                    nc.vector.tensor_scalar_mul(
                        out=aoF[:, b, :], in0=pv[:, b % 2, 0:128],
                        scalar1=rec[:])
                nc.sync.dma_start_transpose(
                    out=ao_prev[:, hp, :].rearrange("p (b m) -> p b m", b=npm),
                    in_=aoF[:])

            for t in range(nmt):
                nj = (t + 1) * npm
                npairs = nj // 2
                m0 = t * mt
                tile_units = []
                # x DMA for the projections woven in the NEXT tile
                if t + 2 < nmt:
                    xm_n = xm_pool.tile([128, et, mt], BF16, tag="xm")
                    xm_tiles[t + 2] = xm_n
                    for c0 in range(0, et, 4):
                        nc.gpsimd.dma_start(
                            xm_n[:, c0:c0 + 4, :],
                            xT_t[:, c0:c0 + 4, (t + 2) * mt:(t + 3) * mt])
                if t + 1 < nmt:
                    q_tiles[t + 1] = qm_pool.tile([128, hl, mt], BF16,
                                                  tag="qm", name="q_sb")
                    tile_units.extend(proj_units(t + 1))

                pending.extend(tile_units)
                ao = ao_pool.tile([128, hl, mt], BF16, tag="ao")
                ao_tiles[t] = ao

                for h in range(hl):
                    flush((t, h))
                    if t > 0 and h == 1:
                        # ao(t-1) is complete only after head 0's evac above.
                        # Interleave output-projection units among the
                        # remaining projection units (roughly 3:1) so their
                        # PSUM-ring dependencies get breathing room.
                        op_units = outproj_units(t - 1, (t, 9))
                        rest = pending[:]
                        pending.clear()
                        k = max(1, len(rest) // max(1, len(op_units)))
                        oi = 0
                        for i, u in enumerate(rest):
                            pending.append(u)
                            if (i + 1) % k == 0 and oi < len(op_units):
                                pending.append(op_units[oi])
                                oi += 1
                        pending.extend(op_units[oi:])
                    rate = (len(pending) / float(npairs * (hl - h))
                            if pending else 0.0)
                    prev = deferred[0]
                    pao_t = {}
                    prev_pairs = len(prev[2]) if prev else 0
                    at2s = []
                    for p in range(npairs):
                        ps2 = psc.tile([128, 2, mt], F32, tag="psc")
                        at2 = att_pool.tile([128, 2, mt], BF16, tag="att")
                        for jj in range(2):
                            j = 2 * p + jj
                            r = j - t * npm
                            c0 = max(r, 0) * 128
                            nc.tensor.matmul(
                                ps2[:, jj, c0:],
                                kT_sb[:, h, j * 128:(j + 1) * 128],
                                q_tiles[t][:, h, c0:], start=True,
                                stop=(r < 0))
                            if r >= 0:  # -1e9 on the strictly-upper diagonal
                                nc.tensor.matmul(
                                    ps2[:, jj, r * 128:(r + 1) * 128],
                                    ident_sb[:], mask_sb[:], start=False,
                                    stop=True)
                        nc.scalar.activation(
                            out=at2[:], in_=ps2[:],
                            func=mybir.ActivationFunctionType.Exp, scale=scale)
                        at2s.append(at2)
                        if prev is not None and p < prev_pairs:
                            emit_attnv_pair(prev[0], prev[1], prev[2][p], p,
                                            pao_t)
                            if p == prev_pairs - 1:
                                # all pairs of the deferred head are in; its
                                # evac can go out now (early for cross-tile)
                                evac_head(prev[0], prev[1], pao_t, prev[3])
                        pull(rate)
                    deferred[0] = (t, h, at2s, ao)
                flush((t, 9))

            # ---------------- epilogue ----------------
            # last head's deferred attention, then the final output projection
            prev = deferred[0]
            pao_t = {}
            for p in range(len(prev[2])):
                emit_attnv_pair(prev[0], prev[1], prev[2][p], p, pao_t)
                if p == len(prev[2]) - 1:
                    evac_head(prev[0], prev[1], pao_t, prev[3])
            for _, fn in outproj_units(nmt - 1, (nmt, 9)):
                fn()

    nc.compile()
    return nc


# ---------------------------------------------------------------------------
# host glue
# ---------------------------------------------------------------------------

def _rope_tables_np(s, d):
    inv_freq = 1.0 / (BASE ** (np.arange(0, d, 2, dtype=np.float32) / d))
    t = np.arange(s, dtype=np.float32)
    freqs = np.outer(t, inv_freq)
    emb = np.concatenate([freqs, freqs], axis=-1)          # [S, D]
    return np.cos(emb).astype(np.float32), np.sin(emb).astype(np.float32)


def make_in_maps(x, Wq, bq, Wk, bk, Wv, bv, Wp, s=S, e=E, hl=HL, d=D,
                 groups=GROUPS, b=B):
    bf = ml_dtypes.bfloat16
    dh = hl * d
    cos, sin = _rope_tables_np(s, d)
    cosT = np.ascontiguousarray(cos.T).astype(bf)           # [D, S]
    sgn = np.concatenate([-np.ones(d // 2), np.ones(d // 2)]).astype(np.float32)
    s2T = (np.ascontiguousarray(sin.T) * sgn[:, None]).astype(bf)
    maskv = np.where(np.arange(128)[:, None] <= np.arange(128)[None, :],
                     np.float32(0), np.float32(-1e9)).astype(bf)
    identv = np.eye(128, dtype=bf)
    in_maps = []
    for bi in range(b):
        xT = np.ascontiguousarray(x[bi].T).astype(bf)       # [E, S]
        for g in range(groups):
            fs = slice(g * dh, (g + 1) * dh)
            # bqk layout: column (which*hl + h) = bias for tensor `which`,
            # head h; columns 2*hl.. are the same rolled by 64 partitions
            bqn = np.concatenate([bq[fs].reshape(hl, 128).T,
                                  bk[fs].reshape(hl, 128).T], axis=1)
            bqkv = np.concatenate([bqn, np.roll(bqn, -64, axis=0)], axis=1)
            in_maps.append({
                "xT": xT,
                "wqT": np.ascontiguousarray(Wq[fs, :].T).astype(bf),
                "wkT": np.ascontiguousarray(Wk[fs, :].T).astype(bf),
                "wvT": np.ascontiguousarray(Wv[fs, :].T).astype(bf),
                "wpT": np.ascontiguousarray(Wp[:, fs].T).astype(bf),
                "bqk": np.ascontiguousarray(bqkv).astype(np.float32),
                "bv": np.ascontiguousarray(bv[fs]).astype(np.float32),
                "cosT": cosT,
                "s2T": np.ascontiguousarray(s2T),
                "mask": maskv,
                "ident": identv,
            })
    return in_maps


_NC_CACHE = {}


def _get_kernel():
    key = "full"
    if key not in _NC_CACHE:
        _NC_CACHE[key] = build_attn_kernel()
    return _NC_CACHE[key]


def _run_axon_cached(nc, in_maps):
    """jit once per process; later kernel() calls reuse the compiled runner."""
    import jax
    from jax.sharding import Mesh, PartitionSpec
    from concourse import bass2jax

    if "runner" not in _NC_CACHE:
        bass2jax.install_neuronx_cc_hook()
        n_cores = len(in_maps)
        partition_name = (nc.partition_id_tensor.name
                          if nc.partition_id_tensor else None)
        in_names, out_names, out_avals, zero_outs = [], [], [], []
        for alloc in nc.m.functions[0].allocations:
            if not isinstance(alloc, mybir.MemoryLocationSet):
                continue
            name = alloc.memorylocations[0].name
            if alloc.kind == "ExternalInput":
                if name != partition_name:
                    in_names.append(name)
            elif alloc.kind == "ExternalOutput":
                out_names.append(name)
                shape = tuple(alloc.tensor_shape)
                dtype = mybir.dt.np(alloc.dtype)
                out_avals.append(jax.core.ShapedArray(shape, dtype))
                zero_outs.append(np.zeros(shape, dtype))
        n_params = len(in_names)
        all_in = list(in_names) + out_names + (
            [partition_name] if partition_name else [])

        def _body(*args):
            operands = list(args)
            if partition_name is not None:
                operands.append(bass2jax.partition_id_tensor())
            outs = bass2jax._bass_exec_p.bind(
                *operands, out_avals=tuple(out_avals),
                in_names=tuple(all_in), out_names=tuple(out_names),
                lowering_input_output_aliases=(), sim_require_finite=True,
                sim_require_nnan=True, nc=nc)
            return tuple(outs)

        devices = jax.devices()[:n_cores]
        mesh = Mesh(np.asarray(devices), ("core",))
        in_specs = (PartitionSpec("core"),) * (n_params + len(out_avals))
        out_specs = (PartitionSpec("core"),) * len(out_names)
        fn = jax.jit(jax.shard_map(_body, mesh=mesh, in_specs=in_specs,
                                   out_specs=out_specs, check_rep=False),
                     keep_unused=True)
        _NC_CACHE["runner"] = (fn, in_names, out_names, out_avals, zero_outs,
                               n_cores)
    fn, in_names, out_names, out_avals, zero_outs, n_cores = _NC_CACHE["runner"]
    concat_in = [np.concatenate([np.asarray(m[n]) for m in in_maps], axis=0)
                 for n in in_names]
    concat_zeros = [np.zeros((n_cores * z.shape[0], *z.shape[1:]), z.dtype)
                    for z in zero_outs]
    outs = fn(*concat_in, *concat_zeros)
    return [{n: np.asarray(outs[i]).reshape(n_cores, *out_avals[i].shape)[c]
             for i, n in enumerate(out_names)} for c in range(n_cores)]


def _run(nc, in_maps):
    from concourse._compat import axon_active
    if axon_active():
        try:
            return _run_axon_cached(nc, in_maps)
        except Exception:
            pass  # fall back to the stock path below
    res = run_bass_kernel_spmd(nc, in_maps, core_ids=list(range(len(in_maps))))
    return res.results


def kernel(x, Wq, bq, Wk, bk, Wv, bv, Wp, bp):
    x = np.asarray(x, dtype=np.float32)
    Wq = np.asarray(Wq, np.float32); bq = np.asarray(bq, np.float32)
    Wk = np.asarray(Wk, np.float32); bk = np.asarray(bk, np.float32)
    Wv = np.asarray(Wv, np.float32); bv = np.asarray(bv, np.float32)
    Wp = np.asarray(Wp, np.float32); bp = np.asarray(bp, np.float32)
    nc = _get_kernel()
    in_maps = make_in_maps(x, Wq, bq, Wk, bk, Wv, bv, Wp)
    results = _run(nc, in_maps)
    y = np.empty((B, S, E), np.float32)
    for bi in range(B):
        acc = results[4 * bi + 0]["yT_p"].astype(np.float32).copy()
        for g in range(1, GROUPS):
            acc += results[4 * bi + g]["yT_p"].astype(np.float32)
        y[bi] = acc.T + bp
    return y


# revision 26
# speedup vs baseline: 1.0785x; 1.0117x over previous
"""Causal multi-head attention (B=2, S=2048, E=2048, H=16, D=128) on 8 TRN2 cores.

Sharding: core c = 4*b + g handles batch b and head-group g (4 heads, feature
slice F = [512g, 512g+512)).  Each core computes q/k/v projections for its
heads, RoPE, causal attention, and a partial output projection
yT_p = Wp[:, F] @ attn_out[F].T.  Host sums the 4 partials per batch and adds
bp.

Schedule: fully software-pipelined around the PE.  The attention j-loop for
tile t weaves in (a) the next tile's q/k/v projection matmuls and (b) the
previous tile's output-projection matmuls, so the in-order PE queue always has
independent work while ScalarE computes exp().

attn@V is computed "flipped" (at stationary, v moving) with a ones column
appended to v, so the softmax denominator falls out of the same matmuls
(column 128 of the PSUM accumulator) instead of costing a second PE pass.
The attention output lands [m, d]; normalization is then a per-partition
scalar multiply on DVE, and a DMA-xbar transpose restores [d, m] for the
output projection.  The causal mask is a 0/1 multiply on DVE (not a PE
matmul).  exp() is done on ScalarE over pairs of score blocks to amortize
instruction overhead.
"""

import math

import ml_dtypes
import numpy as np

import concourse.bass as bass
import concourse.mybir as mybir
import concourse.tile as tile
from concourse import bacc
from concourse.bass_utils import run_bass_kernel_spmd

F32 = mybir.dt.float32
BF16 = mybir.dt.bfloat16

B, S, E, H, D = 2, 2048, 2048, 16, 128
N_CORES = 8
GROUPS = 4          # head-groups per batch
HL = H // GROUPS    # heads per core
BASE = 10000.0


def build_attn_kernel(s=S, e=E, hl=HL, d=D, mt=512, n_cores=N_CORES):
    """One SPMD core program: attention for `hl` heads of one batch."""
    dh = hl * d          # local q/k/v feature width
    et = e // 128        # contraction tiles for the projections
    nmt = s // mt        # m-tiles
    npm = mt // 128      # 128-blocks per m-tile
    ft_out = e // 128    # output g-tiles
    scale = 1.0 / math.sqrt(d)

    nc = bacc.Bacc("TRN2", target_bir_lowering=False, debug=False,
                   num_devices=n_cores)

    xT = nc.dram_tensor("xT", [e, s], BF16, kind="ExternalInput").ap()
    wqT = nc.dram_tensor("wqT", [e, dh], BF16, kind="ExternalInput").ap()
    wkT = nc.dram_tensor("wkT", [e, dh], BF16, kind="ExternalInput").ap()
    wvT = nc.dram_tensor("wvT", [e, dh], BF16, kind="ExternalInput").ap()
    wpT = nc.dram_tensor("wpT", [dh, e], BF16, kind="ExternalInput").ap()
    # bqk columns: [bq | bk | bq rolled by 64 partitions | bk rolled]
    bqk = nc.dram_tensor("bqk", [128, 4 * hl], F32, kind="ExternalInput").ap()
    bv = nc.dram_tensor("bv", [dh], F32, kind="ExternalInput").ap()
    cosT = nc.dram_tensor("cosT", [d, s], BF16, kind="ExternalInput").ap()
    s2T = nc.dram_tensor("s2T", [d, s], BF16, kind="ExternalInput").ap()
    mask = nc.dram_tensor("mask", [128, 128], BF16, kind="ExternalInput").ap()
    ident = nc.dram_tensor("ident", [128, 128], BF16, kind="ExternalInput").ap()
    yT_p = nc.dram_tensor("yT_p", [e, s], BF16, kind="ExternalOutput").ap()

    xT_t = xT.rearrange("(a p) m -> p a m", p=128)
    wq_t = wqT.rearrange("(a p) f -> p a f", p=128)
    wk_t = wkT.rearrange("(a p) f -> p a f", p=128)
    wv_t = wvT.rearrange("(a p) f -> p a f", p=128)

    with tile.TileContext(nc) as tc:
        with (
            tc.tile_pool(name="consts", bufs=1) as consts,
            tc.tile_pool(name="xm", bufs=2) as xm_pool,
            tc.tile_pool(name="kv", bufs=1) as kv_pool,
            tc.tile_pool(name="qm", bufs=2) as qm_pool,
            tc.tile_pool(name="att", bufs=12) as att_pool,
            tc.tile_pool(name="aof", bufs=2) as aof_pool,
            tc.tile_pool(name="ao", bufs=2) as ao_pool,
            tc.tile_pool(name="yo", bufs=3) as yo_pool,
            tc.tile_pool(name="rec", bufs=6) as rec_pool,
            tc.tile_pool(name="pp", bufs=2, space="PSUM") as pp,
            tc.tile_pool(name="psc", bufs=2, space="PSUM") as psc,
            tc.tile_pool(name="pao", bufs=2, space="PSUM") as pao,
        ):
            # ---- startup feed.  sync queue: x tile 0 + small consts + q
            # weights; gpsimd (SWDGE) queue: v/k weights + x tile 1 + p
            # weights.  Chunked so the first projection matmuls can start as
            # soon as the leading chunks land. ----
            # Strict priority order on the two HWDGE queues (they round-robin
            # into the shared DMA engines): criticals first, background last.
            xm_tiles = {}
            xm0 = xm_pool.tile([128, et, mt], BF16, tag="xm")
            xm_tiles[0] = xm0
            wv_sb = consts.tile([128, et, dh], BF16)
            xbounds = [0, 1, 2] + list(range(4, et + 1, 2))
            for c0, c1 in zip(xbounds[:-1], xbounds[1:]):
                nc.sync.dma_start(xm0[:, c0:c1, :], xT_t[:, c0:c1, 0:mt])
                nc.scalar.dma_start(wv_sb[:, c0:c1, :], wv_t[:, c0:c1, :])
            bqk_sb = consts.tile([128, 4 * hl], F32)
            nc.scalar.dma_start(bqk_sb[:], bqk[:])
            bv_sb = consts.tile([128, npm, 128], F32)
            nc.scalar.dma_start(bv_sb[:], bass.AP(
                tensor=bv.tensor, offset=bv.offset, ap=[[0, 128], [1, dh]]))
            mask_sb = consts.tile([128, 128], BF16)
            nc.scalar.dma_start(mask_sb[:], mask[:])
            ident_sb = consts.tile([128, 128], BF16)
            nc.scalar.dma_start(ident_sb[:], ident[:])
            wq_sb = consts.tile([128, et, dh], BF16)
            wk_sb = consts.tile([128, et, dh], BF16)
            for c0 in range(0, et, 2):
                nc.sync.dma_start(wq_sb[:, c0:c0 + 2, :], wq_t[:, c0:c0 + 2, :])
                nc.scalar.dma_start(wk_sb[:, c0:c0 + 2, :], wk_t[:, c0:c0 + 2, :])
            cos_sb = consts.tile([128, s], BF16)
            s2_sb = consts.tile([128, s], BF16)
            nc.sync.dma_start(cos_sb[:], cosT[:])
            nc.sync.dma_start(s2_sb[:], s2T[:])
            # background: x tile 1 (weaves into attention tile 0) and Wp
            xm1 = xm_pool.tile([128, et, mt], BF16, tag="xm")
            xm_tiles[1] = xm1
            for c0 in range(0, et, 4):
                nc.scalar.dma_start(xm1[:, c0:c0 + 4, :],
                                    xT_t[:, c0:c0 + 4, mt:2 * mt])
            wp_sb = consts.tile([128, hl, e], BF16)
            wp_t = wpT.rearrange("(a p) g -> p a g", p=128)
            for hh in range(hl):
                nc.sync.dma_start(wp_sb[:, hh, :], wp_t[:, hh, :])

            kT_sb = kv_pool.tile([128, hl, s], BF16)    # rope'd k, [d, h, n]
            # v with a ones column per (n-block, head): [n_in, j, h, 129]
            v_ext = kv_pool.tile([128, s // 128, hl, 129], BF16)
            nc.vector.memset(v_ext[:, :, :, 128:129], 1.0)
            # zero the score PSUM banks once: paired exp() reads whole banks,
            # including regions no score matmul has written yet
            for _ in range(2):
                zps = psc.tile([128, 2, mt], F32, tag="psc")
                nc.vector.memset(zps[:], 0.0)

            q_tiles = {}
            ao_tiles = {}

            # ---------------- weave machinery ----------------
            pending = []          # list of (deadline, emit_fn); deadline sorts

            def flush(deadline):
                keep = []
                for dl, fn in pending:
                    if dl <= deadline:
                        fn()
                    else:
                        keep.append((dl, fn))
                pending[:] = keep

            pull_acc = [0.0]

            def pull(rate):
                pull_acc[0] += rate
                while pull_acc[0] >= 1.0 and pending:
                    dl, fn = pending.pop(0)
                    fn()
                    pull_acc[0] -= 1.0

            # ---------------- projection units ----------------
            AC = 4                       # contraction chunk per unit

            def proj_units(t):
                """Units for tile t's q/k/v projections (a-chunk major)."""
                units = []
                xm = xm_tiles[t]
                m0 = t * mt
                # v: out rows [m0+nt*128) -> v_ext[:, t*npm+nt, :, 0:128]
                ps_v = {}

                def v_unit(c0, nt):
                    def emit():
                        if c0 == 0:
                            ps_v[nt] = pp.tile([128, npm, 128], F32, tag="pp", name="ps_v")
                        for a in range(c0, c0 + AC):
                            nc.tensor.matmul(
                                ps_v[nt][:], xm[:, a, nt * 128:(nt + 1) * 128],
                                wv_sb[:, a, :], start=(a == 0), stop=(a == et - 1))
                        if c0 + AC == et:
                            j = t * npm + nt
                            nc.vector.tensor_add(out=v_ext[:, j, :, 0:128],
                                                 in0=ps_v[nt][:], in1=bv_sb[:])
                            del ps_v[nt]
                    return emit

                ps_qk = {}

                def qk_unit(which, w_sb, c0, h):
                    def emit():
                        if c0 == 0:
                            ps_qk[(which, h)] = pp.tile([128, mt], F32, tag="pp", name="ps_qk")
                        ps_q = ps_qk[(which, h)]
                        for a in range(c0, c0 + AC):
                            nc.tensor.matmul(
                                ps_q[:], w_sb[:, a, h * 128:(h + 1) * 128],
                                xm[:, a, :], start=(a == 0), stop=(a == et - 1))
                        if c0 + AC == et:
                            bias = bqk_sb[:, which * hl + h:which * hl + h + 1]
                            biasr = bqk_sb[:, 2 * hl + which * hl + h:
                                           2 * hl + which * hl + h + 1]
                            # the three PSUM-reading ops go on DVE (short
                            # queue, fast pp-bank release; the partition-
                            # rolled read is only legal from PSUM); the final
                            # all-SBUF add runs on the idle Pool engine
                            tcos = rec_pool.tile([128, mt], F32, tag="tcos",
                                                 bufs=2)
                            nc.vector.scalar_tensor_tensor(
                                out=tcos[:], in0=ps_q[:], scalar=bias,
                                in1=cos_sb[:, m0:m0 + mt],
                                op0=mybir.AluOpType.add,
                                op1=mybir.AluOpType.mult)
                            u = rec_pool.tile([128, mt], F32, tag="u", bufs=2)
                            nc.vector.scalar_tensor_tensor(
                                out=u[0:64, :], in0=ps_q[64:128, :],
                                scalar=biasr[0:64, :],
                                in1=s2_sb[0:64, m0:m0 + mt],
                                op0=mybir.AluOpType.add,
                                op1=mybir.AluOpType.mult)
                            nc.vector.scalar_tensor_tensor(
                                out=u[64:128, :], in0=ps_q[0:64, :],
                                scalar=biasr[64:128, :],
                                in1=s2_sb[64:128, m0:m0 + mt],
                                op0=mybir.AluOpType.add,
                                op1=mybir.AluOpType.mult)
                            out_ap = (q_tiles[t][:, h, :] if which == 0
                                      else kT_sb[:, h, m0:m0 + mt])
                            nc.gpsimd.tensor_add(out=out_ap, in0=tcos[:],
                                                 in1=u[:])
                            del ps_qk[(which, h)]
                    return emit

                # lagged interleave: exactly 2 chains in flight (pp pool
                # has 2 banks), with consecutive chain-ends a full unit
                # apart so each chain's PSUM-reading tail ops can finish
                # before its bank is reallocated.
                def lace(chains):
                    # chains: list of (deadline, [unit...]); emit with lag
                    out, nu = [], et // AC
                    seq = []
                    for ci, (dl, us) in enumerate(chains):
                        for ui, u_ in enumerate(us):
                            seq.append((2 * ui + 5 * ci, ci, (dl, u_)))
                    seq.sort(key=lambda x: (x[0], x[1]))
                    return [x[2] for x in seq]

                vchains = [((t, 0), [v_unit(c0, nt)
                                     for c0 in range(0, et, AC)])
                           for nt in range(npm)]
                units.extend(lace(vchains))
                for which, w_sb in ((0, wq_sb), (1, wk_sb)):
                    qchains = [((t, h), [qk_unit(which, w_sb, c0, h)
                                         for c0 in range(0, et, AC)])
                               for h in range(hl)]
                    units.extend(lace(qchains))
                return units

            # ---------------- output-projection units ----------------
            def outproj_units(t_prev, deadline):
                units = []
                m0p = t_prev * mt
                ao_prev = ao_tiles[t_prev]
                # woven at tile t_prev+1: Act is exp-loaded at the last tile,
                # so those units evacuate via DVE; the epilogue also spreads
                # its output DMAs over both HWDGE queues
                on_dve = (t_prev == nmt - 2)
                epilogue = (t_prev == nmt - 1)

                def yt_unit(gt):
                    def emit():
                        ps_y = pp.tile([128, mt], F32, tag="pp")
                        for h in range(hl):
                            nc.tensor.matmul(
                                ps_y[:], wp_sb[:, h, gt * 128:(gt + 1) * 128],
                                ao_prev[:, h, :], start=(h == 0),
                                stop=(h == hl - 1))
                        yo = yo_pool.tile([128, mt], BF16, tag="yo")
                        if on_dve or (epilogue and gt % 2):
                            nc.vector.tensor_copy(yo[:], ps_y[:])
                        else:
                            nc.scalar.copy(out=yo[:], in_=ps_y[:])
                        q = (nc.scalar if (epilogue and gt % 2) else nc.sync)
                        q.dma_start(
                            yT_p[gt * 128:(gt + 1) * 128, m0p:m0p + mt], yo[:])
                    return emit

                for gt in range(ft_out):
                    units.append((deadline, yt_unit(gt)))
                return units

            # ---------------- prologue: projections for tile 0 ----------------
            q_tiles[0] = qm_pool.tile([128, hl, mt], BF16, tag="qm", name="q_sb")
            for _, fn in proj_units(0):
                fn()

            # ---------------- main loop ----------------
            # attn@V for head (t,h) is deferred into head (t,h+1)'s pair loop:
            # the PE never waits on exp(), which trails a full head behind.
            deferred = [None]        # (t, h, [at2 per pair], ao tile)
            epilogue_reserve = []

            def emit_attnv_pair(tp, hp, at2_p, p, pao_t):
                for jj in range(2):
                    j = 2 * p + jj
                    r = j - tp * npm
                    for b in range(max(r, 0), npm):
                        half = b // 2
                        if half not in pao_t and j == 0:
                            pao_t[half] = pao.tile([128, 2, 129], F32,
                                                   tag="pao", name="pao_t")
                        # one start=True per PSUM bank: a second start while
                        # the sibling slot's accumulation group is open wipes
                        # the open region (hw bank-scoped start)
                        nc.tensor.matmul(
                            pao_t[half][:, b % 2, :],
                            at2_p[:, jj, b * 128:(b + 1) * 128],
                            v_ext[:, j, hp, :],
                            start=(j == 0 and b % 2 == 0),
                            stop=(j == tp * npm + b),
                            skip_group_check=(b % 2 == 1))

            def evac_head(tp, hp, pao_t, ao_prev):
                # normalize straight out of PSUM + one fused xbar transpose
                aoF = aof_pool.tile([128, npm, 128], BF16, tag="aof",
                                    name="aoF")
                for b in range(npm):
                    pv = pao_t[b // 2]
                    rec = rec_pool.tile([128, 1], F32, tag="rec", name="rec")
                    nc.vector.reciprocal(out=rec[:], in_=pv[:, b % 2, 128:129])
                    nc.vector.tensor_scalar_mul(
                        out=aoF[:, b, :], in0=pv[:, b % 2, 0:128],
                        scalar1=rec[:])
                nc.sync.dma_start_transpose(
                    out=ao_prev[:, hp, :].rearrange("p (b m) -> p b m", b=npm),
                    in_=aoF[:])

            for t in range(nmt):
                nj = (t + 1) * npm
                npairs = nj // 2
                m0 = t * mt
                tile_units = []
                # x DMA for the projections woven in the NEXT tile
                if t + 2 < nmt:
                    xm_n = xm_pool.tile([128, et, mt], BF16, tag="xm")
                    xm_tiles[t + 2] = xm_n
                    for c0 in range(0, et, 4):
                        nc.gpsimd.dma_start(
                            xm_n[:, c0:c0 + 4, :],
                            xT_t[:, c0:c0 + 4, (t + 2) * mt:(t + 3) * mt])
                if t + 1 < nmt:
                    q_tiles[t + 1] = qm_pool.tile([128, hl, mt], BF16,
                                                  tag="qm", name="q_sb")
                    tile_units.extend(proj_units(t + 1))

                pending.extend(tile_units)
                ao = ao_pool.tile([128, hl, mt], BF16, tag="ao")
                ao_tiles[t] = ao

                for h in range(hl):
                    flush((t, h))
                    if t > 0 and h == 1:
                        # ao(t-1) is complete only after head 0's evac above.
                        # Interleave output-projection units among the
                        # remaining projection units (roughly 3:1) so their
                        # PSUM-ring dependencies get breathing room.
                        op_units = outproj_units(t - 1, (t, 9))
                        if t == nmt - 1:
                            epilogue_reserve.extend(op_units[-5:])
                            op_units = op_units[:-5]
                        rest = pending[:]
                        pending.clear()
                        k = max(1, len(rest) // max(1, len(op_units)))
                        oi = 0
                        for i, u in enumerate(rest):
                            pending.append(u)
                            if (i + 1) % k == 0 and oi < len(op_units):
                                pending.append(op_units[oi])
                                oi += 1
                        pending.extend(op_units[oi:])
                    rate = (len(pending) / float(npairs * (hl - h))
                            if pending else 0.0)
                    prev = deferred[0]
                    pao_t = {}
                    prev_pairs = len(prev[2]) if prev else 0
                    at2s = []
                    for p in range(npairs):
                        ps2 = psc.tile([128, 2, mt], F32, tag="psc")
                        at2 = att_pool.tile([128, 2, mt], BF16, tag="att")
                        for jj in range(2):
                            j = 2 * p + jj
                            r = j - t * npm
                            c0 = max(r, 0) * 128
                            nc.tensor.matmul(
                                ps2[:, jj, c0:],
                                kT_sb[:, h, j * 128:(j + 1) * 128],
                                q_tiles[t][:, h, c0:], start=True,
                                stop=(r < 0))
                            if r >= 0:  # -1e9 on the strictly-upper diagonal
                                nc.tensor.matmul(
                                    ps2[:, jj, r * 128:(r + 1) * 128],
                                    ident_sb[:], mask_sb[:], start=False,
                                    stop=True)
                        nc.scalar.activation(
                            out=at2[:], in_=ps2[:],
                            func=mybir.ActivationFunctionType.Exp, scale=scale)
                        at2s.append(at2)
                        if prev is not None and p < prev_pairs:
                            emit_attnv_pair(prev[0], prev[1], prev[2][p], p,
                                            pao_t)
                            if p == prev_pairs - 1:
                                # all pairs of the deferred head are in; its
                                # evac can go out now (early for cross-tile)
                                evac_head(prev[0], prev[1], pao_t, prev[3])
                        pull(rate)
                    deferred[0] = (t, h, at2s, ao)
                flush((t, 9))

            # ---------------- epilogue ----------------
            # last head's deferred attention, then the final output projection
            prev = deferred[0]
            pao_t = {}
            for p in range(len(prev[2])):
                emit_attnv_pair(prev[0], prev[1], prev[2][p], p, pao_t)
                if p == len(prev[2]) - 1:
                    evac_head(prev[0], prev[1], pao_t, prev[3])
            for _, fn in epilogue_reserve:
                fn()
            for _, fn in outproj_units(nmt - 1, (nmt, 9)):
                fn()

    nc.compile()
    return nc


# ---------------------------------------------------------------------------
# host glue
# ---------------------------------------------------------------------------

def _rope_tables_np(s, d):
    inv_freq = 1.0 / (BASE ** (np.arange(0, d, 2, dtype=np.float32) / d))
    t = np.arange(s, dtype=np.float32)
    freqs = np.outer(t, inv_freq)
    emb = np.concatenate([freqs, freqs], axis=-1)          # [S, D]
    return np.cos(emb).astype(np.float32), np.sin(emb).astype(np.float32)


def make_in_maps(x, Wq, bq, Wk, bk, Wv, bv, Wp, s=S, e=E, hl=HL, d=D,
                 groups=GROUPS, b=B):
    bf = ml_dtypes.bfloat16
    dh = hl * d
    cos, sin = _rope_tables_np(s, d)
    cosT = np.ascontiguousarray(cos.T).astype(bf)           # [D, S]
    sgn = np.concatenate([-np.ones(d // 2), np.ones(d // 2)]).astype(np.float32)
    s2T = (np.ascontiguousarray(sin.T) * sgn[:, None]).astype(bf)
    maskv = np.where(np.arange(128)[:, None] <= np.arange(128)[None, :],
                     np.float32(0), np.float32(-1e9)).astype(bf)
    identv = np.eye(128, dtype=bf)
    in_maps = []
    for bi in range(b):
        xT = np.ascontiguousarray(x[bi].T).astype(bf)       # [E, S]
        for g in range(groups):
            fs = slice(g * dh, (g + 1) * dh)
            # bqk layout: column (which*hl + h) = bias for tensor `which`,
            # head h; columns 2*hl.. are the same rolled by 64 partitions
            bqn = np.concatenate([bq[fs].reshape(hl, 128).T,
                                  bk[fs].reshape(hl, 128).T], axis=1)
            bqkv = np.concatenate([bqn, np.roll(bqn, -64, axis=0)], axis=1)
            in_maps.append({
                "xT": xT,
                "wqT": np.ascontiguousarray(Wq[fs, :].T).astype(bf),
                "wkT": np.ascontiguousarray(Wk[fs, :].T).astype(bf),
                "wvT": np.ascontiguousarray(Wv[fs, :].T).astype(bf),
                "wpT": np.ascontiguousarray(Wp[:, fs].T).astype(bf),
                "bqk": np.ascontiguousarray(bqkv).astype(np.float32),
                "bv": np.ascontiguousarray(bv[fs]).astype(np.float32),
                "cosT": cosT,
                "s2T": np.ascontiguousarray(s2T),
                "mask": maskv,
                "ident": identv,
            })
    return in_maps


_NC_CACHE = {}


def _get_kernel():
    key = "full"
    if key not in _NC_CACHE:
        _NC_CACHE[key] = build_attn_kernel()
    return _NC_CACHE[key]


def _run_axon_cached(nc, in_maps):
    """jit once per process; later kernel() calls reuse the compiled runner."""
    import jax
    from jax.sharding import Mesh, PartitionSpec
    from concourse import bass2jax

    if "runner" not in _NC_CACHE:
        bass2jax.install_neuronx_cc_hook()
        n_cores = len(in_maps)
        partition_name = (nc.partition_id_tensor.name
                          if nc.partition_id_tensor else None)
        in_names, out_names, out_avals, zero_outs = [], [], [], []
        for alloc in nc.m.functions[0].allocations:
            if not isinstance(alloc, mybir.MemoryLocationSet):
                continue
            name = alloc.memorylocations[0].name
            if alloc.kind == "ExternalInput":
                if name != partition_name:
                    in_names.append(name)
            elif alloc.kind == "ExternalOutput":
                out_names.append(name)
                shape = tuple(alloc.tensor_shape)
                dtype = mybir.dt.np(alloc.dtype)
                out_avals.append(jax.core.ShapedArray(shape, dtype))
                zero_outs.append(np.zeros(shape, dtype))
        n_params = len(in_names)
        all_in = list(in_names) + out_names + (
            [partition_name] if partition_name else [])

        def _body(*args):
            operands = list(args)
            if partition_name is not None:
                operands.append(bass2jax.partition_id_tensor())
            outs = bass2jax._bass_exec_p.bind(
                *operands, out_avals=tuple(out_avals),
                in_names=tuple(all_in), out_names=tuple(out_names),
                lowering_input_output_aliases=(), sim_require_finite=True,
                sim_require_nnan=True, nc=nc)
            return tuple(outs)

        devices = jax.devices()[:n_cores]
        mesh = Mesh(np.asarray(devices), ("core",))
        in_specs = (PartitionSpec("core"),) * (n_params + len(out_avals))
        out_specs = (PartitionSpec("core"),) * len(out_names)
        fn = jax.jit(jax.shard_map(_body, mesh=mesh, in_specs=in_specs,
                                   out_specs=out_specs, check_rep=False),
                     keep_unused=True)
        _NC_CACHE["runner"] = (fn, in_names, out_names, out_avals, zero_outs,
                               n_cores)
    fn, in_names, out_names, out_avals, zero_outs, n_cores = _NC_CACHE["runner"]
    concat_in = [np.concatenate([np.asarray(m[n]) for m in in_maps], axis=0)
                 for n in in_names]
    concat_zeros = [np.zeros((n_cores * z.shape[0], *z.shape[1:]), z.dtype)
                    for z in zero_outs]
    outs = fn(*concat_in, *concat_zeros)
    return [{n: np.asarray(outs[i]).reshape(n_cores, *out_avals[i].shape)[c]
             for i, n in enumerate(out_names)} for c in range(n_cores)]


def _run(nc, in_maps):
    from concourse._compat import axon_active
    if axon_active():
        try:
            return _run_axon_cached(nc, in_maps)
        except Exception:
            pass  # fall back to the stock path below
    res = run_bass_kernel_spmd(nc, in_maps, core_ids=list(range(len(in_maps))))
    return res.results


def kernel(x, Wq, bq, Wk, bk, Wv, bv, Wp, bp):
    x = np.asarray(x, dtype=np.float32)
    Wq = np.asarray(Wq, np.float32); bq = np.asarray(bq, np.float32)
    Wk = np.asarray(Wk, np.float32); bk = np.asarray(bk, np.float32)
    Wv = np.asarray(Wv, np.float32); bv = np.asarray(bv, np.float32)
    Wp = np.asarray(Wp, np.float32); bp = np.asarray(bp, np.float32)
    nc = _get_kernel()
    in_maps = make_in_maps(x, Wq, bq, Wk, bk, Wv, bv, Wp)
    results = _run(nc, in_maps)
    y = np.empty((B, S, E), np.float32)
    for bi in range(B):
        acc = results[4 * bi + 0]["yT_p"].astype(np.float32).copy()
        for g in range(1, GROUPS):
            acc += results[4 * bi + g]["yT_p"].astype(np.float32)
        y[bi] = acc.T + bp
    return y


# revision 44
# speedup vs baseline: 1.1472x; 1.0637x over previous
"""Causal multi-head attention (B=2, S=2048, E=2048, H=16, D=128) on 8 TRN2 cores.

Sharding: core c = 4*b + g handles batch b and head-group g (4 heads, feature
slice F = [512g, 512g+512)).  Each core computes q/k/v projections for its
heads, RoPE, causal attention, and a partial output projection
yT_p = Wp[:, F] @ attn_out[F].T.  Host sums the 4 partials per batch and adds
bp.

Schedule: fully software-pipelined around the in-order PE queue.  attn@V for
head (t,h) is deferred into head (t,h+1)'s pair loop, so the PE never waits
on ScalarE's exp(); the attention pair-loop additionally weaves in the next
tile's q/k/v projection matmuls and the previous tile's output-projection
matmuls (paced by a deadline-tagged work deque), so the PE stays busy
end-to-end.  Engine assignment keeps every PSUM-evacuation op on a
shallow queue: Act does exp() only (paired score blocks, one instruction per
two j-blocks), DVE does the PSUM-releasing rope/bias/normalize/yo ops, Pool
(gpsimd) does the SBUF-only rope adds and the x-tile SWDGE loads.

attn@V is computed "flipped" (at stationary, v moving) with a ones column
appended to v, so the softmax denominator falls out of the same matmuls
(column 128 of the PSUM accumulator) instead of costing a second PE pass.
Within a shared PSUM bank only the first matmul may use start=True — a
second start while the sibling slot's accumulation group is open wipes the
open region (hardware start is bank-scoped).  The attention output lands
[m, d]; normalization is a per-partition reciprocal+scalar multiply on DVE,
and a fused DMA-xbar block-transpose restores [d, m] for the output
projection.  The causal mask is a 0/1 multiply on DVE with a full head of
slack.  The prologue (tile-0 projections) borrows idle score-PSUM banks so
four projection chains can consume DMA chunks in arrival order.
"""

import math

import ml_dtypes
import numpy as np

import concourse.bass as bass
import concourse.mybir as mybir
import concourse.tile as tile
from concourse import bacc
from concourse.bass_utils import run_bass_kernel_spmd

F32 = mybir.dt.float32
BF16 = mybir.dt.bfloat16

B, S, E, H, D = 2, 2048, 2048, 16, 128
N_CORES = 8
GROUPS = 4          # head-groups per batch
HL = H // GROUPS    # heads per core
BASE = 10000.0


def build_attn_kernel(s=S, e=E, hl=HL, d=D, mt=512, n_cores=N_CORES):
    """One SPMD core program: attention for `hl` heads of one batch."""
    dh = hl * d          # local q/k/v feature width
    et = e // 128        # contraction tiles for the projections
    nmt = s // mt        # m-tiles
    npm = mt // 128      # 128-blocks per m-tile
    ft_out = e // 128    # output g-tiles
    scale = 1.0 / math.sqrt(d)

    nc = bacc.Bacc("TRN2", target_bir_lowering=False, debug=False,
                   num_devices=n_cores)

    xT = nc.dram_tensor("xT", [e, s], BF16, kind="ExternalInput").ap()
    wqT = nc.dram_tensor("wqT", [e, dh], BF16, kind="ExternalInput").ap()
    wkT = nc.dram_tensor("wkT", [e, dh], BF16, kind="ExternalInput").ap()
    wvT = nc.dram_tensor("wvT", [e, dh], BF16, kind="ExternalInput").ap()
    wpT = nc.dram_tensor("wpT", [dh, e], BF16, kind="ExternalInput").ap()
    # bqk columns: [bq | bk | bq rolled by 64 partitions | bk rolled]
    bqk = nc.dram_tensor("bqk", [128, 4 * hl], F32, kind="ExternalInput").ap()
    bv = nc.dram_tensor("bv", [dh], F32, kind="ExternalInput").ap()
    cosT = nc.dram_tensor("cosT", [d, s], BF16, kind="ExternalInput").ap()
    s2T = nc.dram_tensor("s2T", [d, s], BF16, kind="ExternalInput").ap()
    tri = nc.dram_tensor("tri", [128, 128], BF16, kind="ExternalInput").ap()
    yT_p = nc.dram_tensor("yT_p", [e, s], BF16, kind="ExternalOutput").ap()

    xT_t = xT.rearrange("(a p) m -> p a m", p=128)
    wq_t = wqT.rearrange("(a p) f -> p a f", p=128)
    wk_t = wkT.rearrange("(a p) f -> p a f", p=128)
    wv_t = wvT.rearrange("(a p) f -> p a f", p=128)

    with tile.TileContext(nc) as tc:
        with (
            tc.tile_pool(name="consts", bufs=1) as consts,
            tc.tile_pool(name="xm", bufs=2) as xm_pool,
            tc.tile_pool(name="kv", bufs=1) as kv_pool,
            tc.tile_pool(name="qm", bufs=2) as qm_pool,
            tc.tile_pool(name="att", bufs=14) as att_pool,
            tc.tile_pool(name="aof", bufs=2) as aof_pool,
            tc.tile_pool(name="ao", bufs=2) as ao_pool,
            tc.tile_pool(name="yo", bufs=8) as yo_pool,
            tc.tile_pool(name="rec", bufs=6) as rec_pool,
            tc.tile_pool(name="pp", bufs=2, space="PSUM") as pp,
            tc.tile_pool(name="psc", bufs=2, space="PSUM") as psc,
            tc.tile_pool(name="pao", bufs=2, space="PSUM") as pao,
        ):
            # ---- startup feed.  sync queue: x tile 0 + small consts + q
            # weights; gpsimd (SWDGE) queue: v/k weights + x tile 1 + p
            # weights.  Chunked so the first projection matmuls can start as
            # soon as the leading chunks land. ----
            # Strict priority order on the two HWDGE queues (they round-robin
            # into the shared DMA engines): criticals first, background last.
            xm_tiles = {}
            xm0 = xm_pool.tile([128, et, mt], BF16, tag="xm")
            xm_tiles[0] = xm0
            wv_sb = consts.tile([128, et, dh], BF16)
            xbounds = [0, 1, 2] + list(range(4, et + 1, 2))
            for c0, c1 in zip(xbounds[:-1], xbounds[1:]):
                nc.sync.dma_start(xm0[:, c0:c1, :], xT_t[:, c0:c1, 0:mt])
                nc.scalar.dma_start(wv_sb[:, c0:c1, :], wv_t[:, c0:c1, :])
            bqk_sb = consts.tile([128, 4 * hl], F32)
            nc.scalar.dma_start(bqk_sb[:], bqk[:])
            bv_sb = consts.tile([128, npm, 128], F32)
            nc.scalar.dma_start(bv_sb[:], bass.AP(
                tensor=bv.tensor, offset=bv.offset, ap=[[0, 128], [1, dh]]))
            tri_sb = consts.tile([128, 128], BF16)
            nc.scalar.dma_start(tri_sb[:], tri[:])
            wq_sb = consts.tile([128, et, dh], BF16)
            wk_sb = consts.tile([128, et, dh], BF16)
            cos_sb = consts.tile([128, s], BF16)
            s2_sb = consts.tile([128, s], BF16)
            for c0 in range(0, et, 2):
                nc.sync.dma_start(wq_sb[:, c0:c0 + 2, :], wq_t[:, c0:c0 + 2, :])
                nc.scalar.dma_start(wk_sb[:, c0:c0 + 2, :], wk_t[:, c0:c0 + 2, :])
                if c0 == 6:
                    nc.sync.dma_start(cos_sb[:], cosT[:])
                    nc.scalar.dma_start(s2_sb[:], s2T[:])
            # background: x tile 1 (weaves into attention tile 0) and Wp
            xm1 = xm_pool.tile([128, et, mt], BF16, tag="xm")
            xm_tiles[1] = xm1
            for c0 in range(0, et, 4):
                nc.scalar.dma_start(xm1[:, c0:c0 + 4, :],
                                    xT_t[:, c0:c0 + 4, mt:2 * mt])
            wp_sb = consts.tile([128, hl, e], BF16)
            wp_t = wpT.rearrange("(a p) g -> p a g", p=128)
            for hh in range(hl):
                nc.sync.dma_start(wp_sb[:, hh, :], wp_t[:, hh, :])

            kT_sb = kv_pool.tile([128, hl, s], BF16)    # rope'd k, [d, h, n]
            # v with a ones column per (n-block, head): [n_in, j, h, 129]
            v_ext = kv_pool.tile([128, s // 128, hl, 129], BF16)
            nc.vector.memset(v_ext[:, :, :, 128:129], 1.0)
            # zero the score PSUM banks once: paired exp() reads whole banks,
            # including regions no score matmul has written yet
            for _ in range(2):
                zps = psc.tile([128, 2, mt], F32, tag="psc")
                nc.vector.memset(zps[:], 0.0)

            q_tiles = {}
            ao_tiles = {}

            # ---------------- weave machinery ----------------
            pending = []          # list of (deadline, emit_fn); deadline sorts

            def flush(deadline):
                keep = []
                for dl, fn in pending:
                    if dl <= deadline:
                        fn()
                    else:
                        keep.append((dl, fn))
                pending[:] = keep

            pull_acc = [0.0]

            def pull(rate):
                pull_acc[0] += rate
                while pull_acc[0] >= 1.0 and pending:
                    dl, fn = pending.pop(0)
                    fn()
                    pull_acc[0] -= 1.0

            # ---------------- projection units ----------------
            AC = 4                       # contraction chunk per unit

            def proj_units(t):
                """Units for tile t's q/k/v projections (a-chunk major)."""
                units = []
                xm = xm_tiles[t]
                m0 = t * mt
                # v: out rows [m0+nt*128) -> v_ext[:, t*npm+nt, :, 0:128]
                ps_v = {}

                borrow = (t == 0)       # prologue: psc banks are idle
                psc_tiles = {}

                def chain_psum(key, kind):
                    # chains 0,1 of each group -> pp; 2,3 -> a borrowed psc
                    # tile (its two banks hold two independent chains)
                    idx = key[-1] if kind != "v" else key
                    ci = idx % (4 if borrow else 2)
                    if not borrow or ci < 2:
                        shape = [128, npm, 128] if kind == "v" else [128, mt]
                        return pp.tile(shape, F32, tag="pp", name="ps_pr")
                    grp = (kind, key[0] if kind != "v" else 0)
                    if grp not in psc_tiles:
                        psc_tiles[grp] = psc.tile([128, 2, mt], F32,
                                                  tag="psc", name="ps_bor")
                    ap = psc_tiles[grp][:, ci - 2, :]
                    if kind == "v":
                        ap = ap.rearrange("p (b m) -> p b m", b=npm)
                    return ap

                def v_unit(c0, nt):
                    def emit():
                        if c0 == 0:
                            ps_v[nt] = chain_psum(nt, "v")
                        for a in range(c0, c0 + AC):
                            nc.tensor.matmul(
                                ps_v[nt][:], xm[:, a, nt * 128:(nt + 1) * 128],
                                wv_sb[:, a, :], start=(a == 0), stop=(a == et - 1))
                        if c0 + AC == et:
                            j = t * npm + nt
                            nc.vector.tensor_add(out=v_ext[:, j, :, 0:128],
                                                 in0=ps_v[nt][:], in1=bv_sb[:])
                            del ps_v[nt]
                    return emit

                ps_qk = {}

                def qk_unit(which, w_sb, c0, h):
                    def emit():
                        if c0 == 0:
                            ps_qk[(which, h)] = chain_psum((which, h), "qk")
                        ps_q = ps_qk[(which, h)]
                        for a in range(c0, c0 + AC):
                            nc.tensor.matmul(
                                ps_q[:], w_sb[:, a, h * 128:(h + 1) * 128],
                                xm[:, a, :], start=(a == 0), stop=(a == et - 1))
                        if c0 + AC == et:
                            bias = bqk_sb[:, which * hl + h:which * hl + h + 1]
                            biasr = bqk_sb[:, 2 * hl + which * hl + h:
                                           2 * hl + which * hl + h + 1]
                            # the three PSUM-reading ops go on DVE (short
                            # queue, fast pp-bank release; the partition-
                            # rolled read is only legal from PSUM); the final
                            # all-SBUF add runs on the idle Pool engine
                            tcos = rec_pool.tile([128, mt], F32, tag="tcos",
                                                 bufs=2)
                            nc.vector.scalar_tensor_tensor(
                                out=tcos[:], in0=ps_q[:], scalar=bias,
                                in1=cos_sb[:, m0:m0 + mt],
                                op0=mybir.AluOpType.add,
                                op1=mybir.AluOpType.mult)
                            u = rec_pool.tile([128, mt], F32, tag="u", bufs=2)
                            nc.vector.scalar_tensor_tensor(
                                out=u[0:64, :], in0=ps_q[64:128, :],
                                scalar=biasr[0:64, :],
                                in1=s2_sb[0:64, m0:m0 + mt],
                                op0=mybir.AluOpType.add,
                                op1=mybir.AluOpType.mult)
                            nc.vector.scalar_tensor_tensor(
                                out=u[64:128, :], in0=ps_q[0:64, :],
                                scalar=biasr[64:128, :],
                                in1=s2_sb[64:128, m0:m0 + mt],
                                op0=mybir.AluOpType.add,
                                op1=mybir.AluOpType.mult)
                            out_ap = (q_tiles[t][:, h, :] if which == 0
                                      else kT_sb[:, h, m0:m0 + mt])
                            nc.gpsimd.tensor_add(out=out_ap, in0=tcos[:],
                                                 in1=u[:])
                            del ps_qk[(which, h)]
                    return emit

                # lagged interleave: exactly 2 chains in flight (pp pool
                # has 2 banks), with consecutive chain-ends a full unit
                # apart so each chain's PSUM-reading tail ops can finish
                # before its bank is reallocated.
                def lace(chains, lag=5):
                    # chains: list of (deadline, [unit...]); emit with lag
                    seq = []
                    for ci, (dl, us) in enumerate(chains):
                        for ui, u_ in enumerate(us):
                            seq.append((2 * ui + lag * ci, ci, (dl, u_)))
                    seq.sort(key=lambda x: (x[0], x[1]))
                    return [x[2] for x in seq]

                vlag = 1 if borrow else 5
                vchains = [((t, 0), [v_unit(c0, nt)
                                     for c0 in range(0, et, AC)])
                           for nt in range(npm)]
                units.extend(lace(vchains, lag=vlag))
                for which, w_sb in ((0, wq_sb), (1, wk_sb)):
                    qchains = [((t, h), [qk_unit(which, w_sb, c0, h)
                                         for c0 in range(0, et, AC)])
                               for h in range(hl)]
                    units.extend(lace(qchains, lag=vlag))
                return units

            # ---------------- output-projection units ----------------
            def outproj_units(t_prev, deadline, gts=None, force_act=False):
                units = []
                m0p = t_prev * mt
                ao_prev = ao_tiles[t_prev]
                # woven at tile t_prev+1: Act is exp-loaded at the last tile,
                # so those units evacuate via DVE; the epilogue also spreads
                # its output DMAs over both HWDGE queues
                on_dve = (t_prev == nmt - 2) and not force_act
                epilogue = (t_prev == nmt - 1) or force_act

                def yt_unit(gt):
                    def emit():
                        # final outproj: alternate into the pao pool (free
                        # after the last head's evac) for 4-bank pipelining
                        pool = (pao if (t_prev == nmt - 1 and gt % 2)
                                else pp)
                        tg = "pao" if pool is pao else "pp"
                        ps_y = pool.tile([128, mt], F32, tag=tg, name="ps_y")
                        for h in range(hl):
                            nc.tensor.matmul(
                                ps_y[:], wp_sb[:, h, gt * 128:(gt + 1) * 128],
                                ao_prev[:, h, :], start=(h == 0),
                                stop=(h == hl - 1))
                        yo = yo_pool.tile([128, mt], BF16, tag="yo")
                        if on_dve or (epilogue and gt % 2):
                            nc.vector.tensor_copy(yo[:], ps_y[:])
                        else:
                            nc.scalar.copy(out=yo[:], in_=ps_y[:])
                        q = (nc.scalar if (t_prev == nmt - 1 and gt >= ft_out - 2)
                             else nc.sync)
                        q.dma_start(
                            yT_p[gt * 128:(gt + 1) * 128, m0p:m0p + mt], yo[:])
                    return emit

                for gt in (range(ft_out) if gts is None else gts):
                    units.append((deadline, yt_unit(gt)))
                return units

            # ---------------- prologue: projections for tile 0 ----------------
            q_tiles[0] = qm_pool.tile([128, hl, mt], BF16, tag="qm", name="q_sb")
            for _, fn in proj_units(0):
                fn()

            # ---------------- main loop ----------------
            # attn@V for head (t,h) is deferred into head (t,h+1)'s pair loop:
            # the PE never waits on exp(), which trails a full head behind.
            deferred = [None]        # (t, h, [at2 per pair], ao tile)
            epilogue_reserve = []

            def emit_attnv_pair(tp, hp, at2_p, p, pao_t):
                for jj in range(2):
                    j = 2 * p + jj
                    r = j - tp * npm
                    for b in range(max(r, 0), npm):
                        half = b // 2
                        if half not in pao_t and j == 0:
                            pao_t[half] = pao.tile([128, 2, 129], F32,
                                                   tag="pao", name="pao_t")
                        # one start=True per PSUM bank: a second start while
                        # the sibling slot's accumulation group is open wipes
                        # the open region (hw bank-scoped start)
                        nc.tensor.matmul(
                            pao_t[half][:, b % 2, :],
                            at2_p[:, jj, b * 128:(b + 1) * 128],
                            v_ext[:, j, hp, :],
                            start=(j == 0 and b % 2 == 0),
                            stop=(j == tp * npm + b),
                            skip_group_check=(b % 2 == 1))

            def evac_head(tp, hp, pao_t, ao_prev):
                # normalize straight out of PSUM + one fused xbar transpose
                aoF = aof_pool.tile([128, npm, 128], BF16, tag="aof",
                                    name="aoF")
                for b in range(npm):
                    pv = pao_t[b // 2]
                    rec = rec_pool.tile([128, 1], F32, tag="rec", name="rec")
                    nc.vector.reciprocal(out=rec[:], in_=pv[:, b % 2, 128:129])
                    nc.vector.tensor_scalar_mul(
                        out=aoF[:, b, :], in0=pv[:, b % 2, 0:128],
                        scalar1=rec[:])
                nc.sync.dma_start_transpose(
                    out=ao_prev[:, hp, :].rearrange("p (b m) -> p b m", b=npm),
                    in_=aoF[:])

            for t in range(nmt):
                nj = (t + 1) * npm
                npairs = nj // 2
                m0 = t * mt
                tile_units = []
                # x DMA for the projections woven in the NEXT tile
                if t + 2 < nmt:
                    xm_n = xm_pool.tile([128, et, mt], BF16, tag="xm")
                    xm_tiles[t + 2] = xm_n
                    for c0 in range(0, et, 4):
                        nc.gpsimd.dma_start(
                            xm_n[:, c0:c0 + 4, :],
                            xT_t[:, c0:c0 + 4, (t + 2) * mt:(t + 3) * mt])
                if t + 1 < nmt:
                    q_tiles[t + 1] = qm_pool.tile([128, hl, mt], BF16,
                                                  tag="qm", name="q_sb")
                    tile_units.extend(proj_units(t + 1))

                pending.extend(tile_units)
                ao = ao_pool.tile([128, hl, mt], BF16, tag="ao")
                ao_tiles[t] = ao

                for h in range(hl):
                    flush((t, h))
                    if t > 0 and h == 1:
                        # ao(t-1) is complete only after head 0's evac above.
                        # Interleave output-projection units among the
                        # remaining projection units (roughly 3:1) so their
                        # PSUM-ring dependencies get breathing room.
                        if t == nmt - 1:
                            op_units = outproj_units(t - 1, (t, 9),
                                                     gts=range(ft_out - 7))
                            epilogue_reserve.extend(outproj_units(
                                t - 1, (t, 9), gts=range(ft_out - 7, ft_out),
                                force_act=True))
                        else:
                            op_units = outproj_units(t - 1, (t, 9))
                        rest = pending[:]
                        pending.clear()
                        k = max(1, len(rest) // max(1, len(op_units)))
                        oi = 0
                        for i, u in enumerate(rest):
                            pending.append(u)
                            if (i + 1) % k == 0 and oi < len(op_units):
                                pending.append(op_units[oi])
                                oi += 1
                        pending.extend(op_units[oi:])
                    rate = (len(pending) / float(npairs * (hl - h))
                            if pending else 0.0)
                    prev = deferred[0]
                    pao_t = {}
                    prev_pairs = len(prev[2]) if prev else 0
                    at2s = []
                    for p in range(npairs):
                        ps2 = psc.tile([128, 2, mt], F32, tag="psc")
                        at2 = att_pool.tile([128, 2, mt], BF16, tag="att")
                        for jj in range(2):
                            j = 2 * p + jj
                            r = j - t * npm
                            c0 = max(r, 0) * 128
                            nc.tensor.matmul(
                                ps2[:, jj, c0:],
                                kT_sb[:, h, j * 128:(j + 1) * 128],
                                q_tiles[t][:, h, c0:], start=True, stop=True)
                        nc.scalar.activation(
                            out=at2[:], in_=ps2[:],
                            func=mybir.ActivationFunctionType.Exp, scale=scale)
                        for jj in range(2):
                            j = 2 * p + jj
                            r = j - t * npm
                            if r >= 0:  # zero the strictly-upper diagonal;
                                # attnV is a full head away, so this DVE op
                                # is never on the PE critical path
                                nc.vector.tensor_mul(
                                    out=at2[:, jj, r * 128:(r + 1) * 128],
                                    in0=at2[:, jj, r * 128:(r + 1) * 128],
                                    in1=tri_sb[:])
                        at2s.append(at2)
                        if prev is not None and p < prev_pairs:
                            emit_attnv_pair(prev[0], prev[1], prev[2][p], p,
                                            pao_t)
                            if p == prev_pairs - 1:
                                # all pairs of the deferred head are in; its
                                # evac can go out now (early for cross-tile)
                                evac_head(prev[0], prev[1], pao_t, prev[3])
                        pull(rate)
                    deferred[0] = (t, h, at2s, ao)
                flush((t, 9))

            # ---------------- epilogue ----------------
            # last head's deferred attention, then the final output projection
            prev = deferred[0]
            pao_t = {}
            for p in range(len(prev[2])):
                emit_attnv_pair(prev[0], prev[1], prev[2][p], p, pao_t)
                if p == len(prev[2]) - 1:
                    evac_head(prev[0], prev[1], pao_t, prev[3])
            for _, fn in epilogue_reserve:
                fn()
            for _, fn in outproj_units(nmt - 1, (nmt, 9)):
                fn()

    nc.compile()
    return nc


# ---------------------------------------------------------------------------
# host glue
# ---------------------------------------------------------------------------

def _rope_tables_np(s, d):
    inv_freq = 1.0 / (BASE ** (np.arange(0, d, 2, dtype=np.float32) / d))
    t = np.arange(s, dtype=np.float32)
    freqs = np.outer(t, inv_freq)
    emb = np.concatenate([freqs, freqs], axis=-1)          # [S, D]
    return np.cos(emb).astype(np.float32), np.sin(emb).astype(np.float32)


def make_in_maps(x, Wq, bq, Wk, bk, Wv, bv, Wp, s=S, e=E, hl=HL, d=D,
                 groups=GROUPS, b=B):
    bf = ml_dtypes.bfloat16
    dh = hl * d
    cos, sin = _rope_tables_np(s, d)
    cosT = np.ascontiguousarray(cos.T).astype(bf)           # [D, S]
    sgn = np.concatenate([-np.ones(d // 2), np.ones(d // 2)]).astype(np.float32)
    s2T = (np.ascontiguousarray(sin.T) * sgn[:, None]).astype(bf)
    triv = np.where(np.arange(128)[:, None] <= np.arange(128)[None, :],
                    np.float32(1), np.float32(0)).astype(bf)
    in_maps = []
    for bi in range(b):
        xT = np.ascontiguousarray(x[bi].T).astype(bf)       # [E, S]
        for g in range(groups):
            fs = slice(g * dh, (g + 1) * dh)
            # bqk layout: column (which*hl + h) = bias for tensor `which`,
            # head h; columns 2*hl.. are the same rolled by 64 partitions
            bqn = np.concatenate([bq[fs].reshape(hl, 128).T,
                                  bk[fs].reshape(hl, 128).T], axis=1)
            bqkv = np.concatenate([bqn, np.roll(bqn, -64, axis=0)], axis=1)
            in_maps.append({
                "xT": xT,
                "wqT": np.ascontiguousarray(Wq[fs, :].T).astype(bf),
                "wkT": np.ascontiguousarray(Wk[fs, :].T).astype(bf),
                "wvT": np.ascontiguousarray(Wv[fs, :].T).astype(bf),
                "wpT": np.ascontiguousarray(Wp[:, fs].T).astype(bf),
                "bqk": np.ascontiguousarray(bqkv).astype(np.float32),
                "bv": np.ascontiguousarray(bv[fs]).astype(np.float32),
                "cosT": cosT,
                "s2T": np.ascontiguousarray(s2T),
                "tri": triv,
            })
    return in_maps


_NC_CACHE = {}


def _get_kernel():
    key = "full"
    if key not in _NC_CACHE:
        _NC_CACHE[key] = build_attn_kernel()
    return _NC_CACHE[key]


def _run_axon_cached(nc, in_maps):
    """jit once per process; later kernel() calls reuse the compiled runner."""
    import jax
    from jax.sharding import Mesh, PartitionSpec
    from concourse import bass2jax

    if "runner" not in _NC_CACHE:
        bass2jax.install_neuronx_cc_hook()
        n_cores = len(in_maps)
        partition_name = (nc.partition_id_tensor.name
                          if nc.partition_id_tensor else None)
        in_names, out_names, out_avals, zero_outs = [], [], [], []
        for alloc in nc.m.functions[0].allocations:
            if not isinstance(alloc, mybir.MemoryLocationSet):
                continue
            name = alloc.memorylocations[0].name
            if alloc.kind == "ExternalInput":
                if name != partition_name:
                    in_names.append(name)
            elif alloc.kind == "ExternalOutput":
                out_names.append(name)
                shape = tuple(alloc.tensor_shape)
                dtype = mybir.dt.np(alloc.dtype)
                out_avals.append(jax.core.ShapedArray(shape, dtype))
                zero_outs.append(np.zeros(shape, dtype))
        n_params = len(in_names)
        all_in = list(in_names) + out_names + (
            [partition_name] if partition_name else [])

        def _body(*args):
            operands = list(args)
            if partition_name is not None:
                operands.append(bass2jax.partition_id_tensor())
            outs = bass2jax._bass_exec_p.bind(
                *operands, out_avals=tuple(out_avals),
                in_names=tuple(all_in), out_names=tuple(out_names),
                lowering_input_output_aliases=(), sim_require_finite=True,
                sim_require_nnan=True, nc=nc)
            return tuple(outs)

        devices = jax.devices()[:n_cores]
        mesh = Mesh(np.asarray(devices), ("core",))
        in_specs = (PartitionSpec("core"),) * (n_params + len(out_avals))
        out_specs = (PartitionSpec("core"),) * len(out_names)
        fn = jax.jit(jax.shard_map(_body, mesh=mesh, in_specs=in_specs,
                                   out_specs=out_specs, check_rep=False),
                     keep_unused=True)
        _NC_CACHE["runner"] = (fn, in_names, out_names, out_avals, zero_outs,
                               n_cores)
    fn, in_names, out_names, out_avals, zero_outs, n_cores = _NC_CACHE["runner"]
    concat_in = [np.concatenate([np.asarray(m[n]) for m in in_maps], axis=0)
                 for n in in_names]
    concat_zeros = [np.zeros((n_cores * z.shape[0], *z.shape[1:]), z.dtype)
                    for z in zero_outs]
    outs = fn(*concat_in, *concat_zeros)
    return [{n: np.asarray(outs[i]).reshape(n_cores, *out_avals[i].shape)[c]
             for i, n in enumerate(out_names)} for c in range(n_cores)]


def _run(nc, in_maps):
    from concourse._compat import axon_active
    if axon_active():
        try:
            return _run_axon_cached(nc, in_maps)
        except Exception:
            pass  # fall back to the stock path below
    res = run_bass_kernel_spmd(nc, in_maps, core_ids=list(range(len(in_maps))))
    return res.results


def kernel(x, Wq, bq, Wk, bk, Wv, bv, Wp, bp):
    x = np.asarray(x, dtype=np.float32)
    Wq = np.asarray(Wq, np.float32); bq = np.asarray(bq, np.float32)
    Wk = np.asarray(Wk, np.float32); bk = np.asarray(bk, np.float32)
    Wv = np.asarray(Wv, np.float32); bv = np.asarray(bv, np.float32)
    Wp = np.asarray(Wp, np.float32); bp = np.asarray(bp, np.float32)
    nc = _get_kernel()
    in_maps = make_in_maps(x, Wq, bq, Wk, bk, Wv, bv, Wp)
    results = _run(nc, in_maps)
    y = np.empty((B, S, E), np.float32)
    for bi in range(B):
        acc = results[4 * bi + 0]["yT_p"].astype(np.float32).copy()
        for g in range(1, GROUPS):
            acc += results[4 * bi + g]["yT_p"].astype(np.float32)
        y[bi] = acc.T + bp
    return y
